# revision 42
# baseline (speedup 1.0000x reference)
"""CQT (constant-Q transform) + amplitude_to_db kernel for Trainium2.

Full-input contract: kernel(x) takes x [32, 64000] f32 and returns
[32, 84, 126] f32, matching:

    frames = pad(x, n_fft//2)[:, t*HOP + n]          # [B, 126, 16384]
    cr/ci  = frames @ Kr.T / Ki.T                    # [B, 84, 126]
    mag    = sqrt(cr^2 + ci^2)
    out    = amplitude_to_db(mag, ref=max per item, amin=1e-5, top_db=80)

Sharding: pure data parallelism - 4 batch items per NeuronCore on 8 cores.

v5: fp8e4 (e4m3) DoubleRow matmuls; one instruction contracts TWO
128-row K-chunk slots (lhsT [128,2,M], rhs [128,2,N]) in the 504 cycles
a single fp16 matmul costs, halving PE time vs fp16.

Sparsity: CQT kernel support halves per octave and is centered, so for
each K-chunk only a PREFIX of bins (lowest ones) is nonzero. Bins are
packed (re,im)-interleaved along psum partitions so each DoubleRow slot
ships only its active prefix (variable stationary width M_e) - about
4x fewer weight bytes than dense 128-wide slots. m2 = re^2+im^2 is then
formed by a tiny pair-summing matmul (S[p,m]=1 iff p//2==m) on the PE.

Precision: single-rounded fp8 pairs give rel_l2 ~1.8e-2 (gate is 2e-2).
The 16 highest-energy (group, chunk) slots instead use error-feedback
"comp" entries: Q0=q(W/2), Q1=q(W-Q0), x0=q(16x), x1=q(32x-x0) and the
two j-slots compute Q0.T x0 + Q1.T x1, halving both quantization noises
where it matters: measured rel_l2 ~1.0e-2.

Per-bin power-of-2 weight scales keep e4m3 in its normal range; the
epilogue unscales for free via ACT per-partition scale operands. GpSimd
runs ONLY the partition all-reduce + one output DMA (element-wise ops
there force a ~7us mid-kernel GpSimd library swap). DMA issues are
gated in need-order: HBM bandwidth is round-robin across in-flight
queues, so ungated late transfers starve the early ones.
"""

import os
import numpy as np
import ml_dtypes

import concourse.bass as bass
import concourse.mybir as mybir
from concourse import bacc
from concourse import bass_isa
from concourse.bass_utils import run_bass_kernel_spmd

# ---- problem constants (hardcoded; must match the reference) ----
SR = 22050
HOP = 512
N_BINS = 84
BPO = 12
FMIN = 32.70319566257483
AMIN = 1e-5
TOP_DB = 80.0
B = 32
N_SAMP = 64000
N_CORES = 8
NI = B // N_CORES            # items per core = 4
T = 1 + N_SAMP // HOP        # 126 frames
NT = NI * T                  # 504
DB_SCALE = 10.0 / np.log(10.0)
P = 128
SPLIT_BIN = 64               # group A: bins [0,64), group B: bins [64,84)
NBB = N_BINS - SPLIT_BIN     # 20
X_SCALE = 16.0
F8 = ml_dtypes.float8_e4m3   # == mybir.dt.float8e4

SCHEME = os.environ.get("CQT_SCHEME", "hybrid")   # hybrid | comp | pairs
# Skip the final output-DMA semaphore waits (teardown then overlaps the
# output DMA). Measured neutral-to-slightly-worse, so off by default.
NOWAIT = os.environ.get("CQT_NOWAIT", "0") == "1"


def _build_cqt_kernels():
    """Same construction as the reference (nnAudio-style direct CQT bank)."""
    Q = 1.0 / (2.0 ** (1.0 / BPO) - 1.0)
    freqs = FMIN * 2.0 ** (np.arange(N_BINS) / BPO)
    lengths = np.ceil(Q * SR / freqs).astype(int)
    n_fft = int(2 ** np.ceil(np.log2(lengths.max())))
    K = np.zeros((N_BINS, n_fft), dtype=np.complex128)
    for k in range(N_BINS):
        L = int(lengths[k])
        t = np.arange(L) - (L - 1) / 2.0
        kern = np.hanning(L) * np.exp(2j * np.pi * freqs[k] * t / SR)
        kern /= np.abs(kern).sum()
        kern /= np.sqrt(L)
        s = (n_fft - L) // 2
        K[k, s:s + L] = kern
    return K.real.astype(np.float32), K.imag.astype(np.float32), n_fft


Kr, Ki, N_FFT = _build_cqt_kernels()
PAD = N_FFT // 2
FW = (N_SAMP + 2 * PAD) // P      # 628
QW = FW // 4                      # 157
XB = 2 * NI * QW                  # 1256, one x-DMA block (2 phases x NI x QW)
assert (N_SAMP + 2 * PAD) % P == 0 and HOP == 4 * P

_NZ = (np.abs(Kr) + np.abs(Ki)) > 0


def _chunk_range(bins):
    nz = _NZ[bins].any(axis=0)
    idx = np.nonzero(nz)[0]
    return int(idx[0]) // P, int(idx[-1]) // P + 1

_A0, _A1 = _chunk_range(range(0, SPLIT_BIN))
_B0, _B1 = _chunk_range(range(SPLIT_BIN, N_BINS))
CHUNKS_A = list(range(_A0, _A1))   # 90 chunks
CHUNKS_B = list(range(_B0, _B1))   # 4 chunks


def _nbins(kind, c):
    """Active-bin count for (group, chunk); active bins are a prefix of
    the group (lowest bins have the widest support)."""
    lo, hi = (0, SPLIT_BIN) if kind == "A" else (SPLIT_BIN, N_BINS)
    act = np.nonzero(_NZ[lo:hi, c * P:(c + 1) * P].any(axis=1))[0]
    assert len(act) > 0 and act[-1] == len(act) - 1, (kind, c, act)
    return int(len(act))

# per-bin power-of-2 scales: peak |w| lands in [80, 160) (e4m3 max = 240)
_wmax = np.maximum(np.abs(Kr).max(axis=1), np.abs(Ki).max(axis=1))
SBIN = 2.0 ** np.floor(np.log2(160.0 / _wmax))

# comp set: top-16 (group, chunk) by filterbank energy (sim rel_l2 1.02e-2)
if SCHEME == "comp":
    COMP = {("A", c) for c in CHUNKS_A} | {("B", c) for c in CHUNKS_B}
elif SCHEME == "pairs":
    COMP = set()
else:
    COMP = ({("B", 63), ("B", 64)} |
            {("A", c) for c in range(57, 71)})


def _xoff(c, j):
    """Column offset of (chunk, j-variant) in the xt free layout
    (j, r, i, q): off = j*2512 + r*628 + i*157 + q."""
    return j * (4 * NI * QW) + (c % 4) * (NI * QW) + (c // 4)


def _xblk(c, j):
    """Which of the 5 x DMA pieces holds slice (c, j): the j0 phase-0
    and phase-1 halves are separate DMAs so the first matmuls can start
    as soon as ~80KB has landed."""
    if j == 0:
        r = c % 4
        return r if r < 2 else 2
    return 3 + (c % 4) // 2


# x DMA pieces as (col_lo, col_hi) of the xt free layout
XPIECE = [(0, NI * QW), (NI * QW, 2 * NI * QW), (XB, 2 * XB),
          (2 * XB, 3 * XB), (3 * XB, 4 * XB)]
N_XP = len(XPIECE)


def _build_schedule():
    """Entries: dict(kind, comp, s0, s1, m). Ordering: j0-only pair
    entries by x-block, then comp entries (need j1 blocks); B entries
    lead each segment; a few j0 pairs are held back to the end so the
    last B entry retires well before the last A entry."""
    ents = []
    for kind, chunks in (("A", CHUNKS_A), ("B", CHUNKS_B)):
        kord = 0 if kind == "B" else 1
        comp = [c for c in chunks if (kind, c) in COMP]
        rest = [c for c in chunks if (kind, c) not in COMP]
        for c in comp:
            ents.append(dict(kind=kind, comp=True, s0=(c, 0), s1=(c, 1),
                             m=2 * _nbins(kind, c),
                             key=(_xblk(c, 1), kord, c % 4, c // 4)))
        byphase = {}
        for c in rest:
            byphase.setdefault(c % 4, []).append(c)
        leftovers = []
        for r in sorted(byphase):
            lst = sorted(byphase[r])
            while len(lst) >= 2:
                c1, c2 = lst.pop(0), lst.pop(0)
                ents.append(dict(kind=kind, comp=False, s0=(c1, 0), s1=(c2, 0),
                                 m=2 * max(_nbins(kind, c1), _nbins(kind, c2)),
                                 key=(_xblk(c1, 0), kord, c1 % 4, c1 // 4)))
            leftovers += lst
        leftovers.sort(key=lambda c: _xoff(c, 0))
        while len(leftovers) >= 2:
            c1, c2 = leftovers.pop(0), leftovers.pop(0)
            blk = max(_xblk(c1, 0), _xblk(c2, 0))
            ents.append(dict(kind=kind, comp=False, s0=(c1, 0), s1=(c2, 0),
                             m=2 * max(_nbins(kind, c1), _nbins(kind, c2)),
                             key=(blk, kord, 5, 999)))
        if leftovers:   # odd count: upgrade the last single to a comp entry
            c = leftovers[0]
            ents.append(dict(kind=kind, comp=True, s0=(c, 0), s1=(c, 1),
                             m=2 * _nbins(kind, c),
                             key=(_xblk(c, 1), kord, 5, 999)))
    ents.sort(key=lambda e: e["key"])
    # hold back up to 4 j0-only A pairs to the very end (B-drain slack)
    tail = [e for e in ents if e["kind"] == "A" and not e["comp"]
            and e["key"][0] == 1][-4:]
    for e in tail:
        ents.remove(e)
    ents += tail
    assert ents[-1]["kind"] == "A"
    # the first entry of each group carries start=True, so it must cover
    # the group's full partition range (variable-width entries only touch
    # their prefix partitions)
    for kind, full in (("A", P), ("B", 2 * NBB)):
        first = next(e for e in ents if e["kind"] == kind)
        first["m"] = max(first["m"], full)
    return ents


SCHEDULE = _build_schedule()
NE = len(SCHEDULE)

# header: sactA | sactB scale vectors (fp32), S_A and S_B pair-sum
# matrices (bf16), all bitcast into fp8 columns
HDR_SACTA = 0            # 4 cols  (fp32 [128,1])
HDR_SACTB = 4            # 4 cols
HDR_SA = 8               # 128 cols (bf16 [128, 64])
HDR_SB = 136             # 40 cols  (bf16 [.., 20])
HDR = 176

def _mpad(m):
    """DoubleRow fp8 LDWEIGHTS requires the j-slot stride to be even and
    16B-aligned (s3_lw_dual_fp8_restrictions)."""
    return (m + 15) // 16 * 16


WOFF = []
_off = HDR
for _e in SCHEDULE:
    WOFF.append(_off)
    _off += 2 * _mpad(_e["m"])
W_COLS = _off

# weight slab boundaries: by cumulative bytes (finer early)
_bfr = [0.03, 0.08, 0.18, 0.35, 0.6, 1.0]
SLAB_ENDS = []
prev = 0
for f in _bfr:
    target = HDR + f * (W_COLS - HDR)
    e = prev + 1
    while e < NE and WOFF[e] < target:
        e += 1
    e = min(e, NE)
    if e > prev:
        SLAB_ENDS.append(e)
        prev = e
SLAB_ENDS[-1] = NE
N_SLABS = len(SLAB_ENDS)


def _slab_of(e):
    for s, end in enumerate(SLAB_ENDS):
        if e < end:
            return s
    raise IndexError(e)


def _slab_cols(s):
    lo = 0 if s == 0 else WOFF[SLAB_ENDS[s - 1]] if SLAB_ENDS[s - 1] < NE else W_COLS
    hi = WOFF[SLAB_ENDS[s]] if SLAB_ENDS[s] < NE else W_COLS
    return lo, hi


def _pack_weights():
    q = lambda a: a.astype(F8)
    KrT = (Kr * SBIN[:, None]).T.astype(np.float32)   # [N_FFT, 84]
    KiT = (Ki * SBIN[:, None]).T.astype(np.float32)

    def chunk_w(kind, c, m):
        """Interleaved [128, m] block: col 2k = re(bin k), 2k+1 = im."""
        base = 0 if kind == "A" else SPLIT_BIN
        Wc = np.zeros((P, m), np.float32)
        nb = _nbins(kind, c)
        rows = slice(c * P, (c + 1) * P)
        Wc[:, 0:2 * nb:2] = KrT[rows, base:base + nb]
        Wc[:, 1:2 * nb:2] = KiT[rows, base:base + nb]
        return Wc

    w = np.zeros((P, W_COLS), F8)
    for e, ent in enumerate(SCHEDULE):
        base = WOFF[e]
        m = ent["m"]
        mp = _mpad(m)
        if ent["comp"]:
            Wc = chunk_w(ent["kind"], ent["s0"][0], m)
            Q0 = q(Wc * 0.5)
            Q1 = q(Wc - Q0.astype(np.float32))
            w[:, base:base + m] = Q0
            w[:, base + mp:base + mp + m] = Q1
        else:
            w[:, base:base + m] = q(chunk_w(ent["kind"], ent["s0"][0], m))
            w[:, base + mp:base + mp + m] = q(chunk_w(ent["kind"], ent["s1"][0], m))

    sinv = 1.0 / (SBIN * X_SCALE)
    sactA = np.ones(P, np.float32)
    sactA[:] = sinv[np.arange(P) // 2]                  # psA partition 2k/2k+1
    sactB = np.ones(P, np.float32)
    sactB[:2 * NBB] = sinv[SPLIT_BIN + np.arange(2 * NBB) // 2]
    SA = np.zeros((P, SPLIT_BIN), ml_dtypes.bfloat16)
    SA[np.arange(P), np.arange(P) // 2] = 1.0
    SB = np.zeros((P, NBB), ml_dtypes.bfloat16)
    SB[np.arange(2 * NBB), np.arange(2 * NBB) // 2] = 1.0
    wu8 = w.view(np.uint8)
    wu8[:, HDR_SACTA:HDR_SACTA + 4] = sactA.astype("<f4").view(np.uint8).reshape(P, 4)
    wu8[:, HDR_SACTB:HDR_SACTB + 4] = sactB.astype("<f4").view(np.uint8).reshape(P, 4)
    wu8[:, HDR_SA:HDR_SA + 2 * SPLIT_BIN] = SA.view(np.uint8).reshape(P, -1)
    wu8[:, HDR_SB:HDR_SB + 2 * NBB] = SB.view(np.uint8).reshape(P, -1)
    return w


W_NP = _pack_weights()


def pack_x(x):
    """x [32, 64000] f32 -> per-core fp8 packs [4, 128, 1256].

    SBUF xt free layout (j, r, i, q); DMA block k = 2j + r//2.
    x0 = q(16 x); x1 = q(32 x - x0)."""
    xs = np.asarray(x, dtype=np.float32)
    xp = np.pad(xs, ((0, 0), (PAD, PAD)))                 # [32, 80384]
    x0 = (xp * X_SCALE).astype(F8)
    x1 = (2.0 * X_SCALE * xp - x0.astype(np.float32)).astype(F8)
    # sample n = 512 q + 128 r + p  ->  [j, bi, q, r, p]
    X = np.stack([x0, x1]).reshape(2, B, QW, 4, P)
    X = X.reshape(2, B, QW, 2, 2, P)                      # [j, bi, q, rblk, rib, p]
    packs = []
    for core in range(N_CORES):
        blk = X[:, core * NI:(core + 1) * NI]             # [j, i, q, rblk, rib, p]
        arr = blk.transpose(0, 3, 5, 4, 1, 2)             # [j, rblk, p, rib, i, q]
        packs.append(np.ascontiguousarray(arr.reshape(4, P, XB)))
    return packs


def build_program():
    nc = bacc.Bacc("TRN2", target_bir_lowering=False, debug=False,
                   enable_asserts=True)
    f8 = mybir.dt.float8e4
    bf16 = mybir.dt.bfloat16
    f32 = mybir.dt.float32
    DR = mybir.MatmulPerfMode.DoubleRow
    Ln = mybir.ActivationFunctionType.Ln
    Square = mybir.ActivationFunctionType.Square
    Ident = mybir.ActivationFunctionType.Identity

    x_in = nc.dram_tensor("x_in", [4, P, XB], f8, kind="ExternalInput").ap()
    w_in = nc.dram_tensor("w_in", [P, W_COLS], f8, kind="ExternalInput").ap()
    out = nc.dram_tensor("out", [N_BINS, NI, T], f32, kind="ExternalOutput").ap()

    xt = nc.alloc_sbuf_tensor("xt", [P, 4 * XB], f8).ap()
    wt = nc.alloc_sbuf_tensor("wt", [P, W_COLS], f8).ap()
    junk = nc.alloc_sbuf_tensor("junk", [P, 512], f8).ap()
    bufA = nc.alloc_sbuf_tensor("bufA", [P, NT], bf16).ap()
    bufB = nc.alloc_sbuf_tensor("bufB", [2 * NBB, NT], bf16).ap()
    m2c = nc.alloc_sbuf_tensor("m2c", [N_BINS, NT], f32).ap()
    lnm = nc.alloc_sbuf_tensor("lnm", [N_BINS, NT], f32).ap()
    db = nc.alloc_sbuf_tensor("db", [N_BINS, NT], f32).ap()
    r1u = nc.alloc_sbuf_tensor("r1u", [N_BINS, NI], f32).ap()
    rall = nc.alloc_sbuf_tensor("rall", [N_BINS, NI], f32).ap()
    lnr = nc.alloc_sbuf_tensor("lnr", [N_BINS, NI], f32).ap()
    nlnr = nc.alloc_sbuf_tensor("nlnr", [N_BINS, NI], f32).ap()
    lnwarm = nc.alloc_sbuf_tensor("lnwarm", [1, 3], f32).ap()

    psW = nc.alloc_psum_tensor("psW", [P, NT], f32).ap()
    psA = nc.alloc_psum_tensor("psA", [P, NT], f32).ap()
    psB = nc.alloc_psum_tensor("psB", [2 * NBB, NT], f32).ap()
    psM = nc.alloc_psum_tensor("psM", [N_BINS, NT], f32).ap()

    sactA = wt[:, HDR_SACTA:HDR_SACTA + 4].bitcast(f32)    # [128, 1]
    sactB = wt[:, HDR_SACTB:HDR_SACTB + 4].bitcast(f32)
    SAv = wt[:, HDR_SA:HDR_SA + 2 * SPLIT_BIN].bitcast(bf16)   # [128, 64]
    SBv = wt[:, HDR_SB:HDR_SB + 2 * NBB].bitcast(bf16)         # [128, 20]

    s_x = [nc.alloc_semaphore(f"s_x{k}") for k in range(N_XP)]
    s_w = [nc.alloc_semaphore(f"s_w{s}") for s in range(N_SLABS)]
    s_mi = nc.alloc_semaphore("s_mi")
    s_pe = nc.alloc_semaphore("s_pe")   # 1 psB final, 2 psA final, 3 psM final
    s_a = nc.alloc_semaphore("s_a")     # scalar ACT steps
    s_v = nc.alloc_semaphore("s_v")     # vector steps
    s_g = nc.alloc_semaphore("s_g")     # gpsimd steps
    s_out = nc.alloc_semaphore("s_out")
    s_out2 = nc.alloc_semaphore("s_out2")

    xv = xt.rearrange("p (j r i q) -> p j r i q", j=2, r=4, i=NI)

    def rhs_for(ent):
        (c1, j1), (c2, j2) = ent["s0"], ent["s1"]
        o1, o2 = _xoff(c1, j1), _xoff(c2, j2)
        assert o2 > o1, (c1, j1, c2, j2)
        base = xv[:, j1, c1 % 4, :, (c1 // 4):(c1 // 4) + T]   # [128, NI, T]
        u = base.unsqueeze(1)
        u.ap[1] = [o2 - o1, 2]                                  # [128, 2, NI, T]
        return u

    def lhs_for(e):
        m = SCHEDULE[e]["m"]
        lo = WOFF[e]
        u = wt[:, lo:lo + m].unsqueeze(1)
        u.ap[1] = [_mpad(m), 2]       # [128, 2, m] with 16B-aligned j stride
        return u

    def slab_dma(eng, s):
        lo, hi = _slab_cols(s)
        eng.dma_start(wt[:, lo:hi], w_in[:, lo:hi]).then_inc(s_w[s], 16)

    def x_dma(eng, k):
        lo, hi = XPIECE[k]
        if k < 2:
            src = x_in[0][:, lo:hi]
        else:
            src = x_in[k - 1]
        eng.dma_start(xt[:, lo:hi], src).then_inc(s_x[k], 16)

    outf = out.rearrange("k i t -> k (i t)")

    with nc.Block() as block:

        # DMA choreography: HBM bandwidth is shared round-robin across all
        # in-flight hardware queues, so late big transfers are gated behind
        # the early small ones (ungated they starve the early slabs, which
        # stalls the PE and resets the HAM clock ramp).
        @block.sync
        def _(sync):
            x_dma(sync, 0)
            x_dma(sync, 1)
            sync.wait_ge(s_x[1], 16)
            for k in range(2, N_XP):
                x_dma(sync, k)
            sync.wait_ge(s_a, 6)
            sync.dma_start(outf[:, :2 * T], db[:, :2 * T]).then_inc(s_out, 16)
            if not NOWAIT:
                sync.wait_ge(s_out, 16)

        @block.scalar
        def _(scalar):
            # slabs 0-2 are small and needed first: keep them unthrottled
            # (only x block 0 competes); gate the big tail slabs behind them
            for s in range(min(3, N_SLABS)):
                slab_dma(scalar, s)
            if N_SLABS > 3:
                scalar.wait_ge(s_w[2], 16)
                for s in range(3, N_SLABS):
                    slab_dma(scalar, s)
            # preload ACT tables (Ln + Square + Identity) while DMAs fly
            scalar.activation(lnwarm[:, 0:1], nc.const_aps.tensor(1.0, (1, 1)), Ln)
            scalar.activation(lnwarm[:, 1:2], nc.const_aps.tensor(1.0, (1, 1)),
                              Square)
            scalar.activation(lnwarm[:, 2:3], nc.const_aps.tensor(1.0, (1, 1)),
                              Ident)
            scalar.wait_ge(s_pe, 1)
            scalar.activation(bufB[:], psB[:], Square,
                              scale=sactB[:2 * NBB]).then_inc(s_a)     # 1
            scalar.wait_ge(s_pe, 2)
            scalar.activation(bufA[:], psA[:], Square,
                              scale=sactA[:]).then_inc(s_a)            # 2
            scalar.wait_ge(s_v, 1)
            scalar.activation(lnm[:], m2c[:], Ln).then_inc(s_a)        # 3
            scalar.wait_ge(s_g, 1)
            scalar.activation(lnr[:], rall[:], Ln).then_inc(s_a)       # 4
            scalar.wait_ge(s_v, 3)
            for i in range(2):   # db_i = Identity(DB_SCALE*lnm + (-DB_SCALE*lnr_i))
                scalar.activation(db[:, i * T:(i + 1) * T],
                                  lnm[:, i * T:(i + 1) * T], Ident,
                                  bias=nlnr[:, i:i + 1],
                                  scale=float(DB_SCALE)).then_inc(s_a)  # 5, 6

        @block.vector
        def _(vector):
            vector.wait_ge(s_pe, 3)
            vector.tensor_scalar_max(m2c[:], psM[:],
                                     float(AMIN) ** 2).then_inc(s_v)   # 1
            vector.tensor_reduce(r1u[:], psM.rearrange("p (i f) -> p i f", i=NI),
                                 axis=mybir.AxisListType.X,
                                 op=mybir.AluOpType.max).then_inc(s_v)  # 2
            vector.wait_ge(s_a, 4)
            vector.tensor_scalar_mul(nlnr[:], lnr[:],
                                     -float(DB_SCALE)).then_inc(s_v)   # 3
            for i in range(2, 4):
                vector.tensor_scalar(db[:, i * T:(i + 1) * T],
                                     lnm[:, i * T:(i + 1) * T],
                                     lnr[:, i:i + 1], float(DB_SCALE),
                                     mybir.AluOpType.subtract,
                                     mybir.AluOpType.mult)
            vector.drain().then_inc(s_v)                                # 4

        @block.gpsimd
        def _(gpsimd):
            gpsimd.wait_ge(s_v, 2)
            gpsimd.partition_all_reduce(rall[:], r1u[:], channels=N_BINS,
                                        reduce_op=bass_isa.ReduceOp.max
                                        ).then_inc(s_g)                # 1
            gpsimd.wait_ge(s_v, 4)
            gpsimd.dma_start(outf[:, 2 * T:], db[:, 2 * T:]).then_inc(s_out2, 16)
            if not NOWAIT:
                gpsimd.wait_ge(s_out2, 16)

        @block.tensor
        def _(tensor):
            # HAM warmup on whatever garbage sits in junk's SBUF region:
            # psW is never read and every real accumulation opens with
            # start=True, so the values are irrelevant - starting the PE
            # immediately buys clock-ramp time. Tapered sizes keep the PE
            # continuously busy up to the input-DMA arrival instant with
            # fine granularity at the handoff (an idle gap restarts the
            # clock ramp).
            for n in (NT, NT, NT, NT, NT // 2, NT // 2, NT // 4, NT // 4):
                tensor.matmul(psW[:, :n], lhsT=junk[:, :P], rhs=junk[:, :n],
                              start=True, stop=True)
            waited = set()

            def need(sem):
                if id(sem) not in waited:
                    tensor.wait_ge(sem, 16)
                    waited.add(id(sem))

            na = sum(1 for e in SCHEDULE if e["kind"] == "A")
            nb = NE - na
            na_seen = nb_seen = 0
            for e, ent in enumerate(SCHEDULE):
                need(s_x[_xblk(*ent["s0"])])
                need(s_x[_xblk(*ent["s1"])])
                need(s_w[_slab_of(e)])
                m = ent["m"]
                ps = psA if ent["kind"] == "A" else psB
                first = (na_seen == 0) if ent["kind"] == "A" else (nb_seen == 0)
                last = (na_seen == na - 1) if ent["kind"] == "A" \
                    else (nb_seen == nb - 1)
                tensor.matmul(ps[:m], lhsT=lhs_for(e), rhs=rhs_for(ent),
                              start=first, stop=last, perf_mode=DR,
                              skip_group_check=True)
                if ent["kind"] == "A":
                    na_seen += 1
                    if na_seen == na:
                        tensor.drain().then_inc(s_pe, 1)       # 2 (B first)
                else:
                    nb_seen += 1
                    if nb_seen == nb:
                        tensor.drain().then_inc(s_pe, 1)       # 1
            # pair-sum matmuls: psM[k] = buf[2k] + buf[2k+1] = re^2 + im^2
            tensor.wait_ge(s_a, 1)
            tensor.matmul(psM[SPLIT_BIN:], lhsT=SBv[:2 * NBB], rhs=bufB[:],
                          start=True, stop=True, skip_group_check=True)
            tensor.wait_ge(s_a, 2)
            tensor.matmul(psM[:SPLIT_BIN], lhsT=SAv[:], rhs=bufA[:],
                          start=True, stop=True, skip_group_check=True)
            tensor.drain().then_inc(s_pe, 1)                   # 3

    nc.compile()
    return nc


_PROGRAM = None


def _get_program():
    global _PROGRAM
    if _PROGRAM is None:
        _PROGRAM = build_program()
    return _PROGRAM


def run(x, **spmd_kwargs):
    """Run on 8 NeuronCores; returns (output [32, 84, 126] f32, results)."""
    nc = _get_program()
    packs = pack_x(x)
    in_maps = [{"x_in": packs[i], "w_in": W_NP} for i in range(N_CORES)]
    res = run_bass_kernel_spmd(nc, in_maps, core_ids=list(range(N_CORES)),
                               **spmd_kwargs)
    out = np.concatenate([res.results[i]["out"].transpose(1, 0, 2)
                          for i in range(N_CORES)], axis=0)
    return np.ascontiguousarray(out.astype(np.float32)), res


def kernel(x):
    return run(x)[0]


# revision 43
# speedup vs baseline: 1.1747x; 1.1747x over previous
"""CQT (constant-Q transform) + amplitude_to_db kernel for Trainium2.

Full-input contract: kernel(x) takes x [32, 64000] f32 and returns
[32, 84, 126] f32, matching:

    frames = pad(x, n_fft//2)[:, t*HOP + n]          # [B, 126, 16384]
    cr/ci  = frames @ Kr.T / Ki.T                    # [B, 84, 126]
    mag    = sqrt(cr^2 + ci^2)
    out    = amplitude_to_db(mag, ref=max per item, amin=1e-5, top_db=80)

Sharding: pure data parallelism - 4 batch items per NeuronCore on 8 cores.

v5: fp8e4 (e4m3) DoubleRow matmuls; one instruction contracts TWO
128-row K-chunk slots (lhsT [128,2,M], rhs [128,2,N]) in the 504 cycles
a single fp16 matmul costs, halving PE time vs fp16.

Sparsity: CQT kernel support halves per octave and is centered, so for
each K-chunk only a PREFIX of bins (lowest ones) is nonzero. Bins are
packed (re,im)-interleaved along psum partitions so each DoubleRow slot
ships only its active prefix (variable stationary width M_e) - about
4x fewer weight bytes than dense 128-wide slots. m2 = re^2+im^2 is then
formed by a tiny pair-summing matmul (S[p,m]=1 iff p//2==m) on the PE.

Precision: single-rounded fp8 pairs give rel_l2 ~1.8e-2 (gate is 2e-2).
The 16 highest-energy (group, chunk) slots instead use error-feedback
"comp" entries: Q0=q(W/2), Q1=q(W-Q0), x0=q(16x), x1=q(32x-x0) and the
two j-slots compute Q0.T x0 + Q1.T x1, halving both quantization noises
where it matters: measured rel_l2 ~1.0e-2.

Per-bin power-of-2 weight scales keep e4m3 in its normal range; the
epilogue unscales for free via ACT per-partition scale operands. GpSimd
runs ONLY the partition all-reduce + one output DMA (element-wise ops
there force a ~7us mid-kernel GpSimd library swap). DMA issues are
gated in need-order: HBM bandwidth is round-robin across in-flight
queues, so ungated late transfers starve the early ones.
"""

import os
import numpy as np
import ml_dtypes

import concourse.bass as bass
import concourse.mybir as mybir
from concourse import bacc
from concourse import bass_isa
from concourse.bass_utils import run_bass_kernel_spmd

# ---- problem constants (hardcoded; must match the reference) ----
SR = 22050
HOP = 512
N_BINS = 84
BPO = 12
FMIN = 32.70319566257483
AMIN = 1e-5
TOP_DB = 80.0
B = 32
N_SAMP = 64000
N_CORES = 8
NI = B // N_CORES            # items per core = 4
T = 1 + N_SAMP // HOP        # 126 frames
NT = NI * T                  # 504
DB_SCALE = 10.0 / np.log(10.0)
P = 128
SPLIT_BIN = 64               # group A: bins [0,64), group B: bins [64,84)
NBB = N_BINS - SPLIT_BIN     # 20
X_SCALE = 16.0
F8 = ml_dtypes.float8_e4m3   # == mybir.dt.float8e4

SCHEME = os.environ.get("CQT_SCHEME", "hybrid")   # hybrid | comp | pairs
# Skip the final output-DMA semaphore waits (teardown then overlaps the
# output DMA). Measured neutral-to-slightly-worse, so off by default.
NOWAIT = os.environ.get("CQT_NOWAIT", "0") == "1"


def _build_cqt_kernels():
    """Same construction as the reference (nnAudio-style direct CQT bank)."""
    Q = 1.0 / (2.0 ** (1.0 / BPO) - 1.0)
    freqs = FMIN * 2.0 ** (np.arange(N_BINS) / BPO)
    lengths = np.ceil(Q * SR / freqs).astype(int)
    n_fft = int(2 ** np.ceil(np.log2(lengths.max())))
    K = np.zeros((N_BINS, n_fft), dtype=np.complex128)
    for k in range(N_BINS):
        L = int(lengths[k])
        t = np.arange(L) - (L - 1) / 2.0
        kern = np.hanning(L) * np.exp(2j * np.pi * freqs[k] * t / SR)
        kern /= np.abs(kern).sum()
        kern /= np.sqrt(L)
        s = (n_fft - L) // 2
        K[k, s:s + L] = kern
    return K.real.astype(np.float32), K.imag.astype(np.float32), n_fft


Kr, Ki, N_FFT = _build_cqt_kernels()
PAD = N_FFT // 2
FW = (N_SAMP + 2 * PAD) // P      # 628
QW = FW // 4                      # 157
XB = 2 * NI * QW                  # 1256, one x-DMA block (2 phases x NI x QW)
assert (N_SAMP + 2 * PAD) % P == 0 and HOP == 4 * P

_NZ = (np.abs(Kr) + np.abs(Ki)) > 0


def _chunk_range(bins):
    nz = _NZ[bins].any(axis=0)
    idx = np.nonzero(nz)[0]
    return int(idx[0]) // P, int(idx[-1]) // P + 1

_A0, _A1 = _chunk_range(range(0, SPLIT_BIN))
_B0, _B1 = _chunk_range(range(SPLIT_BIN, N_BINS))
CHUNKS_A = list(range(_A0, _A1))   # 90 chunks
CHUNKS_B = list(range(_B0, _B1))   # 4 chunks


def _nbins(kind, c):
    """Active-bin count for (group, chunk); active bins are a prefix of
    the group (lowest bins have the widest support)."""
    lo, hi = (0, SPLIT_BIN) if kind == "A" else (SPLIT_BIN, N_BINS)
    act = np.nonzero(_NZ[lo:hi, c * P:(c + 1) * P].any(axis=1))[0]
    assert len(act) > 0 and act[-1] == len(act) - 1, (kind, c, act)
    return int(len(act))

# per-bin power-of-2 scales: peak |w| lands in [80, 160) (e4m3 max = 240)
_wmax = np.maximum(np.abs(Kr).max(axis=1), np.abs(Ki).max(axis=1))
SBIN = 2.0 ** np.floor(np.log2(160.0 / _wmax))

# comp set: top-16 (group, chunk) by filterbank energy (sim rel_l2 1.02e-2)
if SCHEME == "comp":
    COMP = {("A", c) for c in CHUNKS_A} | {("B", c) for c in CHUNKS_B}
elif SCHEME == "pairs":
    COMP = set()
else:
    COMP = ({("B", 63), ("B", 64)} |
            {("A", c) for c in range(57, 71)})


def _xoff(c, j):
    """Column offset of (chunk, j-variant) in the xt free layout
    (j, r, i, q): off = j*2512 + r*628 + i*157 + q."""
    return j * (4 * NI * QW) + (c % 4) * (NI * QW) + (c // 4)


def _xblk(c, j):
    """Which of the 5 x DMA pieces holds slice (c, j): the j0 phase-0
    and phase-1 halves are separate DMAs so the first matmuls can start
    as soon as ~80KB has landed."""
    if j == 0:
        r = c % 4
        return r if r < 2 else 2
    return 3 + (c % 4) // 2


# x DMA pieces as (col_lo, col_hi) of the xt free layout
XPIECE = [(0, NI * QW), (NI * QW, 2 * NI * QW), (XB, 2 * XB),
          (2 * XB, 3 * XB), (3 * XB, 4 * XB)]
N_XP = len(XPIECE)


def _build_schedule():
    """Entries: dict(kind, comp, s0, s1, m). Ordering: j0-only pair
    entries by x-block, then comp entries (need j1 blocks); B entries
    lead each segment; a few j0 pairs are held back to the end so the
    last B entry retires well before the last A entry."""
    ents = []
    for kind, chunks in (("A", CHUNKS_A), ("B", CHUNKS_B)):
        kord = 0 if kind == "B" else 1
        comp = [c for c in chunks if (kind, c) in COMP]
        rest = [c for c in chunks if (kind, c) not in COMP]
        for c in comp:
            ents.append(dict(kind=kind, comp=True, s0=(c, 0), s1=(c, 1),
                             m=2 * _nbins(kind, c),
                             key=(_xblk(c, 1), kord, c % 4, c // 4)))
        byphase = {}
        for c in rest:
            byphase.setdefault(c % 4, []).append(c)
        leftovers = []
        for r in sorted(byphase):
            lst = sorted(byphase[r])
            while len(lst) >= 2:
                c1, c2 = lst.pop(0), lst.pop(0)
                ents.append(dict(kind=kind, comp=False, s0=(c1, 0), s1=(c2, 0),
                                 m=2 * max(_nbins(kind, c1), _nbins(kind, c2)),
                                 key=(_xblk(c1, 0), kord, c1 % 4, c1 // 4)))
            leftovers += lst
        leftovers.sort(key=lambda c: _xoff(c, 0))
        while len(leftovers) >= 2:
            c1, c2 = leftovers.pop(0), leftovers.pop(0)
            blk = max(_xblk(c1, 0), _xblk(c2, 0))
            ents.append(dict(kind=kind, comp=False, s0=(c1, 0), s1=(c2, 0),
                             m=2 * max(_nbins(kind, c1), _nbins(kind, c2)),
                             key=(blk, kord, 5, 999)))
        if leftovers:   # odd count: upgrade the last single to a comp entry
            c = leftovers[0]
            ents.append(dict(kind=kind, comp=True, s0=(c, 0), s1=(c, 1),
                             m=2 * _nbins(kind, c),
                             key=(_xblk(c, 1), kord, 5, 999)))
    ents.sort(key=lambda e: e["key"])
    # hold back up to 4 j0-only A pairs to the very end (B-drain slack)
    tail = [e for e in ents if e["kind"] == "A" and not e["comp"]
            and e["key"][0] == 1][-4:]
    for e in tail:
        ents.remove(e)
    ents += tail
    assert ents[-1]["kind"] == "A"
    # the first entry of each group carries start=True, so it must cover
    # the group's full partition range (variable-width entries only touch
    # their prefix partitions)
    for kind, full in (("A", P), ("B", 2 * NBB)):
        first = next(e for e in ents if e["kind"] == kind)
        first["m"] = max(first["m"], full)
    return ents


SCHEDULE = _build_schedule()
NE = len(SCHEDULE)

# header: sactA | sactB scale vectors (fp32), S_A and S_B pair-sum
# matrices (bf16), all bitcast into fp8 columns
HDR_SACTA = 0            # 4 cols  (fp32 [128,1])
HDR_SACTB = 4            # 4 cols
HDR_SA = 8               # 128 cols (bf16 [128, 64])
HDR_SB = 136             # 40 cols  (bf16 [.., 20])
HDR = 176

def _mpad(m):
    """DoubleRow fp8 LDWEIGHTS requires the j-slot stride to be even and
    16B-aligned (s3_lw_dual_fp8_restrictions)."""
    return (m + 15) // 16 * 16


WOFF = []
_off = HDR
for _e in SCHEDULE:
    WOFF.append(_off)
    _off += 2 * _mpad(_e["m"])
W_COLS = _off

# weight slab boundaries: by cumulative bytes (finer early)
_bfr = [0.03, 0.08, 0.18, 0.35, 0.6, 1.0]
SLAB_ENDS = []
prev = 0
for f in _bfr:
    target = HDR + f * (W_COLS - HDR)
    e = prev + 1
    while e < NE and WOFF[e] < target:
        e += 1
    e = min(e, NE)
    if e > prev:
        SLAB_ENDS.append(e)
        prev = e
SLAB_ENDS[-1] = NE
N_SLABS = len(SLAB_ENDS)


def _slab_of(e):
    for s, end in enumerate(SLAB_ENDS):
        if e < end:
            return s
    raise IndexError(e)


def _slab_cols(s):
    lo = 0 if s == 0 else WOFF[SLAB_ENDS[s - 1]] if SLAB_ENDS[s - 1] < NE else W_COLS
    hi = WOFF[SLAB_ENDS[s]] if SLAB_ENDS[s] < NE else W_COLS
    return lo, hi


def _pack_weights():
    q = lambda a: a.astype(F8)
    KrT = (Kr * SBIN[:, None]).T.astype(np.float32)   # [N_FFT, 84]
    KiT = (Ki * SBIN[:, None]).T.astype(np.float32)

    def chunk_w(kind, c, m):
        """Interleaved [128, m] block: col 2k = re(bin k), 2k+1 = im."""
        base = 0 if kind == "A" else SPLIT_BIN
        Wc = np.zeros((P, m), np.float32)
        nb = _nbins(kind, c)
        rows = slice(c * P, (c + 1) * P)
        Wc[:, 0:2 * nb:2] = KrT[rows, base:base + nb]
        Wc[:, 1:2 * nb:2] = KiT[rows, base:base + nb]
        return Wc

    w = np.zeros((P, W_COLS), F8)
    for e, ent in enumerate(SCHEDULE):
        base = WOFF[e]
        m = ent["m"]
        mp = _mpad(m)
        if ent["comp"]:
            Wc = chunk_w(ent["kind"], ent["s0"][0], m)
            Q0 = q(Wc * 0.5)
            Q1 = q(Wc - Q0.astype(np.float32))
            w[:, base:base + m] = Q0
            w[:, base + mp:base + mp + m] = Q1
        else:
            w[:, base:base + m] = q(chunk_w(ent["kind"], ent["s0"][0], m))
            w[:, base + mp:base + mp + m] = q(chunk_w(ent["kind"], ent["s1"][0], m))

    sinv = 1.0 / (SBIN * X_SCALE)
    sactA = np.ones(P, np.float32)
    sactA[:] = sinv[np.arange(P) // 2]                  # psA partition 2k/2k+1
    sactB = np.ones(P, np.float32)
    sactB[:2 * NBB] = sinv[SPLIT_BIN + np.arange(2 * NBB) // 2]
    SA = np.zeros((P, SPLIT_BIN), ml_dtypes.bfloat16)
    SA[np.arange(P), np.arange(P) // 2] = 1.0
    SB = np.zeros((P, NBB), ml_dtypes.bfloat16)
    SB[np.arange(2 * NBB), np.arange(2 * NBB) // 2] = 1.0
    wu8 = w.view(np.uint8)
    wu8[:, HDR_SACTA:HDR_SACTA + 4] = sactA.astype("<f4").view(np.uint8).reshape(P, 4)
    wu8[:, HDR_SACTB:HDR_SACTB + 4] = sactB.astype("<f4").view(np.uint8).reshape(P, 4)
    wu8[:, HDR_SA:HDR_SA + 2 * SPLIT_BIN] = SA.view(np.uint8).reshape(P, -1)
    wu8[:, HDR_SB:HDR_SB + 2 * NBB] = SB.view(np.uint8).reshape(P, -1)
    return w


W_NP = _pack_weights()


def pack_x(x):
    """x [32, 64000] f32 -> per-core fp8 packs [4, 128, 1256].

    SBUF xt free layout (j, r, i, q); DMA block k = 2j + r//2.
    x0 = q(16 x); x1 = q(32 x - x0)."""
    xs = np.asarray(x, dtype=np.float32)
    xp = np.pad(xs, ((0, 0), (PAD, PAD)))                 # [32, 80384]
    x0 = (xp * X_SCALE).astype(F8)
    x1 = (2.0 * X_SCALE * xp - x0.astype(np.float32)).astype(F8)
    # sample n = 512 q + 128 r + p  ->  [j, bi, q, r, p]
    X = np.stack([x0, x1]).reshape(2, B, QW, 4, P)
    X = X.reshape(2, B, QW, 2, 2, P)                      # [j, bi, q, rblk, rib, p]
    packs = []
    for core in range(N_CORES):
        blk = X[:, core * NI:(core + 1) * NI]             # [j, i, q, rblk, rib, p]
        arr = blk.transpose(0, 3, 5, 4, 1, 2)             # [j, rblk, p, rib, i, q]
        packs.append(np.ascontiguousarray(arr.reshape(4, P, XB)))
    return packs


def build_program():
    nc = bacc.Bacc("TRN2", target_bir_lowering=False, debug=False,
                   enable_asserts=True)
    f8 = mybir.dt.float8e4
    bf16 = mybir.dt.bfloat16
    f32 = mybir.dt.float32
    DR = mybir.MatmulPerfMode.DoubleRow
    Ln = mybir.ActivationFunctionType.Ln
    Square = mybir.ActivationFunctionType.Square
    Ident = mybir.ActivationFunctionType.Identity

    x_in = nc.dram_tensor("x_in", [4, P, XB], f8, kind="ExternalInput").ap()
    w_in = nc.dram_tensor("w_in", [P, W_COLS], f8, kind="ExternalInput").ap()
    out = nc.dram_tensor("out", [N_BINS, NI, T], f32, kind="ExternalOutput").ap()

    xt = nc.alloc_sbuf_tensor("xt", [P, 4 * XB], f8).ap()
    wt = nc.alloc_sbuf_tensor("wt", [P, W_COLS], f8).ap()
    junk = nc.alloc_sbuf_tensor("junk", [P, 512], f8).ap()
    bufA = nc.alloc_sbuf_tensor("bufA", [P, NT], bf16).ap()
    bufB = nc.alloc_sbuf_tensor("bufB", [2 * NBB, NT], bf16).ap()
    m2c = nc.alloc_sbuf_tensor("m2c", [N_BINS, NT], f32).ap()
    lnm = nc.alloc_sbuf_tensor("lnm", [N_BINS, NT], f32).ap()
    db = nc.alloc_sbuf_tensor("db", [N_BINS, NT], f32).ap()
    r1u = nc.alloc_sbuf_tensor("r1u", [N_BINS, NI], f32).ap()
    rall = nc.alloc_sbuf_tensor("rall", [N_BINS, NI], f32).ap()
    lnr = nc.alloc_sbuf_tensor("lnr", [N_BINS, NI], f32).ap()
    nlnr = nc.alloc_sbuf_tensor("nlnr", [N_BINS, NI], f32).ap()
    lnwarm = nc.alloc_sbuf_tensor("lnwarm", [1, 3], f32).ap()

    psW = nc.alloc_psum_tensor("psW", [P, NT], f32).ap()
    psA = nc.alloc_psum_tensor("psA", [P, NT], f32).ap()
    psB = nc.alloc_psum_tensor("psB", [2 * NBB, NT], f32).ap()
    psM = nc.alloc_psum_tensor("psM", [N_BINS, NT], f32).ap()

    sactA = wt[:, HDR_SACTA:HDR_SACTA + 4].bitcast(f32)    # [128, 1]
    sactB = wt[:, HDR_SACTB:HDR_SACTB + 4].bitcast(f32)
    SAv = wt[:, HDR_SA:HDR_SA + 2 * SPLIT_BIN].bitcast(bf16)   # [128, 64]
    SBv = wt[:, HDR_SB:HDR_SB + 2 * NBB].bitcast(bf16)         # [128, 20]

    s_x = [nc.alloc_semaphore(f"s_x{k}") for k in range(N_XP)]
    s_w = [nc.alloc_semaphore(f"s_w{s}") for s in range(N_SLABS)]
    s_mi = nc.alloc_semaphore("s_mi")
    s_pe = nc.alloc_semaphore("s_pe")   # 1 psB final, 2 psA final, 3 psM final
    s_a = nc.alloc_semaphore("s_a")     # scalar ACT steps
    s_v = nc.alloc_semaphore("s_v")     # vector steps
    s_g = nc.alloc_semaphore("s_g")     # gpsimd steps
    s_out = nc.alloc_semaphore("s_out")
    s_out2 = nc.alloc_semaphore("s_out2")

    xv = xt.rearrange("p (j r i q) -> p j r i q", j=2, r=4, i=NI)

    def rhs_for(ent):
        (c1, j1), (c2, j2) = ent["s0"], ent["s1"]
        o1, o2 = _xoff(c1, j1), _xoff(c2, j2)
        assert o2 > o1, (c1, j1, c2, j2)
        base = xv[:, j1, c1 % 4, :, (c1 // 4):(c1 // 4) + T]   # [128, NI, T]
        u = base.unsqueeze(1)
        u.ap[1] = [o2 - o1, 2]                                  # [128, 2, NI, T]
        return u

    def lhs_for(e):
        m = SCHEDULE[e]["m"]
        lo = WOFF[e]
        u = wt[:, lo:lo + m].unsqueeze(1)
        u.ap[1] = [_mpad(m), 2]       # [128, 2, m] with 16B-aligned j stride
        return u

    def slab_dma(eng, s):
        lo, hi = _slab_cols(s)
        eng.dma_start(wt[:, lo:hi], w_in[:, lo:hi]).then_inc(s_w[s], 16)

    def x_dma(eng, k):
        lo, hi = XPIECE[k]
        if k < 2:
            src = x_in[0][:, lo:hi]
        else:
            src = x_in[k - 1]
        eng.dma_start(xt[:, lo:hi], src).then_inc(s_x[k], 16)

    outf = out.rearrange("k i t -> k (i t)")

    with nc.Block() as block:

        # DMA choreography: HBM bandwidth is shared round-robin across all
        # in-flight hardware queues, so late big transfers are gated behind
        # the early small ones (ungated they starve the early slabs, which
        # stalls the PE and resets the HAM clock ramp).
        @block.sync
        def _(sync):
            x_dma(sync, 0)
            x_dma(sync, 1)
            sync.wait_ge(s_x[1], 16)
            for k in range(2, N_XP):
                x_dma(sync, k)
            sync.wait_ge(s_a, 6)
            sync.dma_start(outf[:, :2 * T], db[:, :2 * T]).then_inc(s_out, 16)
            if not NOWAIT:
                sync.wait_ge(s_out, 16)

        @block.scalar
        def _(scalar):
            # slabs 0-2 are small and needed first: keep them unthrottled
            # (only x block 0 competes); gate the big tail slabs behind them
            for s in range(min(3, N_SLABS)):
                slab_dma(scalar, s)
            if N_SLABS > 3:
                scalar.wait_ge(s_w[2], 16)
                for s in range(3, N_SLABS):
                    slab_dma(scalar, s)
            # preload ACT tables (Ln + Square + Identity) while DMAs fly
            scalar.activation(lnwarm[:, 0:1], nc.const_aps.tensor(1.0, (1, 1)), Ln)
            scalar.activation(lnwarm[:, 1:2], nc.const_aps.tensor(1.0, (1, 1)),
                              Square)
            scalar.activation(lnwarm[:, 2:3], nc.const_aps.tensor(1.0, (1, 1)),
                              Ident)
            scalar.wait_ge(s_pe, 1)
            scalar.activation(bufB[:], psB[:], Square,
                              scale=sactB[:2 * NBB]).then_inc(s_a)     # 1
            scalar.wait_ge(s_pe, 2)
            scalar.activation(bufA[:], psA[:], Square,
                              scale=sactA[:]).then_inc(s_a)            # 2
            scalar.wait_ge(s_v, 1)
            scalar.activation(lnm[:], m2c[:], Ln).then_inc(s_a)        # 3
            scalar.wait_ge(s_g, 1)
            scalar.activation(lnr[:], rall[:], Ln).then_inc(s_a)       # 4
            scalar.wait_ge(s_v, 3)
            for i in range(2):   # db_i = Identity(DB_SCALE*lnm + (-DB_SCALE*lnr_i))
                scalar.activation(db[:, i * T:(i + 1) * T],
                                  lnm[:, i * T:(i + 1) * T], Ident,
                                  bias=nlnr[:, i:i + 1],
                                  scale=float(DB_SCALE)).then_inc(s_a)  # 5, 6

        @block.vector
        def _(vector):
            vector.wait_ge(s_pe, 3)
            vector.tensor_scalar_max(m2c[:], psM[:],
                                     float(AMIN) ** 2).then_inc(s_v)   # 1
            vector.tensor_reduce(r1u[:], psM.rearrange("p (i f) -> p i f", i=NI),
                                 axis=mybir.AxisListType.X,
                                 op=mybir.AluOpType.max).then_inc(s_v)  # 2
            vector.wait_ge(s_a, 4)
            vector.tensor_scalar_mul(nlnr[:], lnr[:],
                                     -float(DB_SCALE)).then_inc(s_v)   # 3
            for i in range(2, 4):
                vector.tensor_scalar(db[:, i * T:(i + 1) * T],
                                     lnm[:, i * T:(i + 1) * T],
                                     lnr[:, i:i + 1], float(DB_SCALE),
                                     mybir.AluOpType.subtract,
                                     mybir.AluOpType.mult)
            vector.drain().then_inc(s_v)                                # 4

        @block.gpsimd
        def _(gpsimd):
            gpsimd.wait_ge(s_v, 2)
            gpsimd.partition_all_reduce(rall[:], r1u[:], channels=N_BINS,
                                        reduce_op=bass_isa.ReduceOp.max
                                        ).then_inc(s_g)                # 1
            gpsimd.wait_ge(s_v, 4)
            gpsimd.dma_start(outf[:, 2 * T:], db[:, 2 * T:]).then_inc(s_out2, 16)
            if not NOWAIT:
                gpsimd.wait_ge(s_out2, 16)

        @block.tensor
        def _(tensor):
            # HAM warmup on whatever garbage sits in junk's SBUF region:
            # psW is never read and every real accumulation opens with
            # start=True, so the values are irrelevant - starting the PE
            # immediately buys clock-ramp time
            for _ in range(5):
                tensor.matmul(psW[:], lhsT=junk[:, :P], rhs=junk[:, :NT],
                              start=True, stop=True)
            waited = set()

            def need(sem):
                if id(sem) not in waited:
                    tensor.wait_ge(sem, 16)
                    waited.add(id(sem))

            na = sum(1 for e in SCHEDULE if e["kind"] == "A")
            nb = NE - na
            na_seen = nb_seen = 0
            for e, ent in enumerate(SCHEDULE):
                need(s_x[_xblk(*ent["s0"])])
                need(s_x[_xblk(*ent["s1"])])
                need(s_w[_slab_of(e)])
                m = ent["m"]
                ps = psA if ent["kind"] == "A" else psB
                first = (na_seen == 0) if ent["kind"] == "A" else (nb_seen == 0)
                last = (na_seen == na - 1) if ent["kind"] == "A" \
                    else (nb_seen == nb - 1)
                tensor.matmul(ps[:m], lhsT=lhs_for(e), rhs=rhs_for(ent),
                              start=first, stop=last, perf_mode=DR,
                              skip_group_check=True)
                if ent["kind"] == "A":
                    na_seen += 1
                    if na_seen == na:
                        tensor.drain().then_inc(s_pe, 1)       # 2 (B first)
                else:
                    nb_seen += 1
                    if nb_seen == nb:
                        tensor.drain().then_inc(s_pe, 1)       # 1
            # pair-sum matmuls: psM[k] = buf[2k] + buf[2k+1] = re^2 + im^2
            tensor.wait_ge(s_a, 1)
            tensor.matmul(psM[SPLIT_BIN:], lhsT=SBv[:2 * NBB], rhs=bufB[:],
                          start=True, stop=True, skip_group_check=True)
            tensor.wait_ge(s_a, 2)
            tensor.matmul(psM[:SPLIT_BIN], lhsT=SAv[:], rhs=bufA[:],
                          start=True, stop=True, skip_group_check=True)
            tensor.drain().then_inc(s_pe, 1)                   # 3

    nc.compile()
    return nc


_PROGRAM = None


def _get_program():
    global _PROGRAM
    if _PROGRAM is None:
        _PROGRAM = build_program()
    return _PROGRAM


def run(x, **spmd_kwargs):
    """Run on 8 NeuronCores; returns (output [32, 84, 126] f32, results)."""
    nc = _get_program()
    packs = pack_x(x)
    in_maps = [{"x_in": packs[i], "w_in": W_NP} for i in range(N_CORES)]
    res = run_bass_kernel_spmd(nc, in_maps, core_ids=list(range(N_CORES)),
                               **spmd_kwargs)
    out = np.concatenate([res.results[i]["out"].transpose(1, 0, 2)
                          for i in range(N_CORES)], axis=0)
    return np.ascontiguousarray(out.astype(np.float32)), res


def kernel(x):
    return run(x)[0]


# revision 44
# speedup vs baseline: 1.1926x; 1.0152x over previous
"""CQT (constant-Q transform) + amplitude_to_db kernel for Trainium2.

Full-input contract: kernel(x) takes x [32, 64000] f32 and returns
[32, 84, 126] f32, matching:

    frames = pad(x, n_fft//2)[:, t*HOP + n]          # [B, 126, 16384]
    cr/ci  = frames @ Kr.T / Ki.T                    # [B, 84, 126]
    mag    = sqrt(cr^2 + ci^2)
    out    = amplitude_to_db(mag, ref=max per item, amin=1e-5, top_db=80)

Sharding: pure data parallelism - 4 batch items per NeuronCore on 8 cores.

v5: fp8e4 (e4m3) DoubleRow matmuls; one instruction contracts TWO
128-row K-chunk slots (lhsT [128,2,M], rhs [128,2,N]) in the 504 cycles
a single fp16 matmul costs, halving PE time vs fp16.

Sparsity: CQT kernel support halves per octave and is centered, so for
each K-chunk only a PREFIX of bins (lowest ones) is nonzero. Bins are
packed (re,im)-interleaved along psum partitions so each DoubleRow slot
ships only its active prefix (variable stationary width M_e) - about
4x fewer weight bytes than dense 128-wide slots. m2 = re^2+im^2 is then
formed by a tiny pair-summing matmul (S[p,m]=1 iff p//2==m) on the PE.

Precision: single-rounded fp8 pairs give rel_l2 ~1.8e-2 (gate is 2e-2).
The 16 highest-energy (group, chunk) slots instead use error-feedback
"comp" entries: Q0=q(W/2), Q1=q(W-Q0), x0=q(16x), x1=q(32x-x0) and the
two j-slots compute Q0.T x0 + Q1.T x1, halving both quantization noises
where it matters: measured rel_l2 ~1.0e-2.

Per-bin power-of-2 weight scales keep e4m3 in its normal range; the
epilogue unscales for free via ACT per-partition scale operands. GpSimd
runs ONLY the partition all-reduce + one output DMA (element-wise ops
there force a ~7us mid-kernel GpSimd library swap). DMA issues are
gated in need-order: HBM bandwidth is round-robin across in-flight
queues, so ungated late transfers starve the early ones.
"""

import os
import numpy as np
import ml_dtypes

import concourse.bass as bass
import concourse.mybir as mybir
from concourse import bacc
from concourse import bass_isa
from concourse.bass_utils import run_bass_kernel_spmd

# ---- problem constants (hardcoded; must match the reference) ----
SR = 22050
HOP = 512
N_BINS = 84
BPO = 12
FMIN = 32.70319566257483
AMIN = 1e-5
TOP_DB = 80.0
B = 32
N_SAMP = 64000
N_CORES = 8
NI = B // N_CORES            # items per core = 4
T = 1 + N_SAMP // HOP        # 126 frames
NT = NI * T                  # 504
DB_SCALE = 10.0 / np.log(10.0)
P = 128
SPLIT_BIN = 64               # group A: bins [0,64), group B: bins [64,84)
NBB = N_BINS - SPLIT_BIN     # 20
X_SCALE = 16.0
F8 = ml_dtypes.float8_e4m3   # == mybir.dt.float8e4

SCHEME = os.environ.get("CQT_SCHEME", "hybrid")   # hybrid | comp | pairs
# Skip the final output-DMA semaphore waits (teardown then overlaps the
# output DMA). Measured neutral-to-slightly-worse, so off by default.
NOWAIT = os.environ.get("CQT_NOWAIT", "0") == "1"


def _build_cqt_kernels():
    """Same construction as the reference (nnAudio-style direct CQT bank)."""
    Q = 1.0 / (2.0 ** (1.0 / BPO) - 1.0)
    freqs = FMIN * 2.0 ** (np.arange(N_BINS) / BPO)
    lengths = np.ceil(Q * SR / freqs).astype(int)
    n_fft = int(2 ** np.ceil(np.log2(lengths.max())))
    K = np.zeros((N_BINS, n_fft), dtype=np.complex128)
    for k in range(N_BINS):
        L = int(lengths[k])
        t = np.arange(L) - (L - 1) / 2.0
        kern = np.hanning(L) * np.exp(2j * np.pi * freqs[k] * t / SR)
        kern /= np.abs(kern).sum()
        kern /= np.sqrt(L)
        s = (n_fft - L) // 2
        K[k, s:s + L] = kern
    return K.real.astype(np.float32), K.imag.astype(np.float32), n_fft


Kr, Ki, N_FFT = _build_cqt_kernels()
PAD = N_FFT // 2
FW = (N_SAMP + 2 * PAD) // P      # 628
QW = FW // 4                      # 157
XB = 2 * NI * QW                  # 1256, one x-DMA block (2 phases x NI x QW)
assert (N_SAMP + 2 * PAD) % P == 0 and HOP == 4 * P

_NZ = (np.abs(Kr) + np.abs(Ki)) > 0


def _chunk_range(bins):
    nz = _NZ[bins].any(axis=0)
    idx = np.nonzero(nz)[0]
    return int(idx[0]) // P, int(idx[-1]) // P + 1

_A0, _A1 = _chunk_range(range(0, SPLIT_BIN))
_B0, _B1 = _chunk_range(range(SPLIT_BIN, N_BINS))
CHUNKS_A = list(range(_A0, _A1))   # 90 chunks
CHUNKS_B = list(range(_B0, _B1))   # 4 chunks


def _nbins(kind, c):
    """Active-bin count for (group, chunk); active bins are a prefix of
    the group (lowest bins have the widest support)."""
    lo, hi = (0, SPLIT_BIN) if kind == "A" else (SPLIT_BIN, N_BINS)
    act = np.nonzero(_NZ[lo:hi, c * P:(c + 1) * P].any(axis=1))[0]
    assert len(act) > 0 and act[-1] == len(act) - 1, (kind, c, act)
    return int(len(act))

# per-bin power-of-2 scales: peak |w| lands in [80, 160) (e4m3 max = 240)
_wmax = np.maximum(np.abs(Kr).max(axis=1), np.abs(Ki).max(axis=1))
SBIN = 2.0 ** np.floor(np.log2(160.0 / _wmax))

# comp set: top-16 (group, chunk) by filterbank energy (sim rel_l2 1.02e-2)
if SCHEME == "comp":
    COMP = {("A", c) for c in CHUNKS_A} | {("B", c) for c in CHUNKS_B}
elif SCHEME == "pairs":
    COMP = set()
else:
    COMP = ({("B", 63), ("B", 64)} |
            {("A", c) for c in range(57, 71)})


def _xoff(c, j):
    """Column offset of (chunk, j-variant) in the xt free layout
    (j, r, i, q): off = j*2512 + r*628 + i*157 + q."""
    return j * (4 * NI * QW) + (c % 4) * (NI * QW) + (c // 4)


def _xblk(c, j):
    """Which of the 5 x DMA pieces holds slice (c, j): the j0 phase-0
    and phase-1 halves are separate DMAs so the first matmuls can start
    as soon as ~80KB has landed."""
    if j == 0:
        r = c % 4
        return r if r < 2 else 2
    return 3 + (c % 4) // 2


# x DMA pieces as (col_lo, col_hi) of the xt free layout
XPIECE = [(0, NI * QW), (NI * QW, 2 * NI * QW), (XB, 2 * XB),
          (2 * XB, 3 * XB), (3 * XB, 4 * XB)]
N_XP = len(XPIECE)


def _build_schedule():
    """Entries: dict(kind, comp, s0, s1, m). Ordering: j0-only pair
    entries by x-block, then comp entries (need j1 blocks); B entries
    lead each segment; a few j0 pairs are held back to the end so the
    last B entry retires well before the last A entry."""
    ents = []
    for kind, chunks in (("A", CHUNKS_A), ("B", CHUNKS_B)):
        kord = 0 if kind == "B" else 1
        comp = [c for c in chunks if (kind, c) in COMP]
        rest = [c for c in chunks if (kind, c) not in COMP]
        for c in comp:
            ents.append(dict(kind=kind, comp=True, s0=(c, 0), s1=(c, 1),
                             m=2 * _nbins(kind, c),
                             key=(_xblk(c, 1), kord, c % 4, c // 4)))
        byphase = {}
        for c in rest:
            byphase.setdefault(c % 4, []).append(c)
        leftovers = []
        for r in sorted(byphase):
            lst = sorted(byphase[r])
            while len(lst) >= 2:
                c1, c2 = lst.pop(0), lst.pop(0)
                ents.append(dict(kind=kind, comp=False, s0=(c1, 0), s1=(c2, 0),
                                 m=2 * max(_nbins(kind, c1), _nbins(kind, c2)),
                                 key=(_xblk(c1, 0), kord, c1 % 4, c1 // 4)))
            leftovers += lst
        leftovers.sort(key=lambda c: _xoff(c, 0))
        while len(leftovers) >= 2:
            c1, c2 = leftovers.pop(0), leftovers.pop(0)
            blk = max(_xblk(c1, 0), _xblk(c2, 0))
            ents.append(dict(kind=kind, comp=False, s0=(c1, 0), s1=(c2, 0),
                             m=2 * max(_nbins(kind, c1), _nbins(kind, c2)),
                             key=(blk, kord, 5, 999)))
        if leftovers:   # odd count: upgrade the last single to a comp entry
            c = leftovers[0]
            ents.append(dict(kind=kind, comp=True, s0=(c, 0), s1=(c, 1),
                             m=2 * _nbins(kind, c),
                             key=(_xblk(c, 1), kord, 5, 999)))
    ents.sort(key=lambda e: e["key"])
    # hold back up to 4 j0-only A pairs to the very end (B-drain slack)
    tail = [e for e in ents if e["kind"] == "A" and not e["comp"]
            and e["key"][0] == 1][-4:]
    for e in tail:
        ents.remove(e)
    ents += tail
    assert ents[-1]["kind"] == "A"
    # the first entry of each group carries start=True, so it must cover
    # the group's full partition range (variable-width entries only touch
    # their prefix partitions)
    for kind, full in (("A", P), ("B", 2 * NBB)):
        first = next(e for e in ents if e["kind"] == kind)
        first["m"] = max(first["m"], full)
    return ents


SCHEDULE = _build_schedule()
NE = len(SCHEDULE)

# header: sactA | sactB scale vectors (fp32), S_A and S_B pair-sum
# matrices (bf16), all bitcast into fp8 columns
HDR_SACTA = 0            # 4 cols  (fp32 [128,1])
HDR_SACTB = 4            # 4 cols
HDR_SA = 8               # 128 cols (bf16 [128, 64])
HDR_SB = 136             # 40 cols  (bf16 [.., 20])
HDR = 176

def _mpad(m):
    """DoubleRow fp8 LDWEIGHTS requires the j-slot stride to be even and
    16B-aligned (s3_lw_dual_fp8_restrictions)."""
    return (m + 15) // 16 * 16


WOFF = []
_off = HDR
for _e in SCHEDULE:
    WOFF.append(_off)
    _off += 2 * _mpad(_e["m"])
W_COLS = _off

# weight slab boundaries: by cumulative bytes (finer early)
_bfr = [0.03, 0.08, 0.18, 0.35, 0.6, 1.0]
SLAB_ENDS = []
prev = 0
for f in _bfr:
    target = HDR + f * (W_COLS - HDR)
    e = prev + 1
    while e < NE and WOFF[e] < target:
        e += 1
    e = min(e, NE)
    if e > prev:
        SLAB_ENDS.append(e)
        prev = e
SLAB_ENDS[-1] = NE
N_SLABS = len(SLAB_ENDS)


def _slab_of(e):
    for s, end in enumerate(SLAB_ENDS):
        if e < end:
            return s
    raise IndexError(e)


def _slab_cols(s):
    lo = 0 if s == 0 else WOFF[SLAB_ENDS[s - 1]] if SLAB_ENDS[s - 1] < NE else W_COLS
    hi = WOFF[SLAB_ENDS[s]] if SLAB_ENDS[s] < NE else W_COLS
    return lo, hi


def _pack_weights():
    q = lambda a: a.astype(F8)
    KrT = (Kr * SBIN[:, None]).T.astype(np.float32)   # [N_FFT, 84]
    KiT = (Ki * SBIN[:, None]).T.astype(np.float32)

    def chunk_w(kind, c, m):
        """Interleaved [128, m] block: col 2k = re(bin k), 2k+1 = im."""
        base = 0 if kind == "A" else SPLIT_BIN
        Wc = np.zeros((P, m), np.float32)
        nb = _nbins(kind, c)
        rows = slice(c * P, (c + 1) * P)
        Wc[:, 0:2 * nb:2] = KrT[rows, base:base + nb]
        Wc[:, 1:2 * nb:2] = KiT[rows, base:base + nb]
        return Wc

    w = np.zeros((P, W_COLS), F8)
    for e, ent in enumerate(SCHEDULE):
        base = WOFF[e]
        m = ent["m"]
        mp = _mpad(m)
        if ent["comp"]:
            Wc = chunk_w(ent["kind"], ent["s0"][0], m)
            Q0 = q(Wc * 0.5)
            Q1 = q(Wc - Q0.astype(np.float32))
            w[:, base:base + m] = Q0
            w[:, base + mp:base + mp + m] = Q1
        else:
            w[:, base:base + m] = q(chunk_w(ent["kind"], ent["s0"][0], m))
            w[:, base + mp:base + mp + m] = q(chunk_w(ent["kind"], ent["s1"][0], m))

    sinv = 1.0 / (SBIN * X_SCALE)
    sactA = np.ones(P, np.float32)
    sactA[:] = sinv[np.arange(P) // 2]                  # psA partition 2k/2k+1
    sactB = np.ones(P, np.float32)
    sactB[:2 * NBB] = sinv[SPLIT_BIN + np.arange(2 * NBB) // 2]
    SA = np.zeros((P, SPLIT_BIN), ml_dtypes.bfloat16)
    SA[np.arange(P), np.arange(P) // 2] = 1.0
    SB = np.zeros((P, NBB), ml_dtypes.bfloat16)
    SB[np.arange(2 * NBB), np.arange(2 * NBB) // 2] = 1.0
    wu8 = w.view(np.uint8)
    wu8[:, HDR_SACTA:HDR_SACTA + 4] = sactA.astype("<f4").view(np.uint8).reshape(P, 4)
    wu8[:, HDR_SACTB:HDR_SACTB + 4] = sactB.astype("<f4").view(np.uint8).reshape(P, 4)
    wu8[:, HDR_SA:HDR_SA + 2 * SPLIT_BIN] = SA.view(np.uint8).reshape(P, -1)
    wu8[:, HDR_SB:HDR_SB + 2 * NBB] = SB.view(np.uint8).reshape(P, -1)
    return w


W_NP = _pack_weights()


def pack_x(x):
    """x [32, 64000] f32 -> per-core fp8 packs [4, 128, 1256].

    SBUF xt free layout (j, r, i, q); DMA block k = 2j + r//2.
    x0 = q(16 x); x1 = q(32 x - x0)."""
    xs = np.asarray(x, dtype=np.float32)
    xp = np.pad(xs, ((0, 0), (PAD, PAD)))                 # [32, 80384]
    x0 = (xp * X_SCALE).astype(F8)
    x1 = (2.0 * X_SCALE * xp - x0.astype(np.float32)).astype(F8)
    # sample n = 512 q + 128 r + p  ->  [j, bi, q, r, p]
    X = np.stack([x0, x1]).reshape(2, B, QW, 4, P)
    X = X.reshape(2, B, QW, 2, 2, P)                      # [j, bi, q, rblk, rib, p]
    packs = []
    for core in range(N_CORES):
        blk = X[:, core * NI:(core + 1) * NI]             # [j, i, q, rblk, rib, p]
        arr = blk.transpose(0, 3, 5, 4, 1, 2)             # [j, rblk, p, rib, i, q]
        packs.append(np.ascontiguousarray(arr.reshape(4, P, XB)))
    return packs


def build_program():
    nc = bacc.Bacc("TRN2", target_bir_lowering=False, debug=False,
                   enable_asserts=True)
    f8 = mybir.dt.float8e4
    bf16 = mybir.dt.bfloat16
    f32 = mybir.dt.float32
    DR = mybir.MatmulPerfMode.DoubleRow
    Ln = mybir.ActivationFunctionType.Ln
    Square = mybir.ActivationFunctionType.Square
    Ident = mybir.ActivationFunctionType.Identity

    x_in = nc.dram_tensor("x_in", [4, P, XB], f8, kind="ExternalInput").ap()
    w_in = nc.dram_tensor("w_in", [P, W_COLS], f8, kind="ExternalInput").ap()
    out = nc.dram_tensor("out", [N_BINS, NI, T], f32, kind="ExternalOutput").ap()

    xt = nc.alloc_sbuf_tensor("xt", [P, 4 * XB], f8).ap()
    wt = nc.alloc_sbuf_tensor("wt", [P, W_COLS], f8).ap()
    junk = nc.alloc_sbuf_tensor("junk", [P, 512], f8).ap()
    bufA = nc.alloc_sbuf_tensor("bufA", [P, NT], bf16).ap()
    bufB = nc.alloc_sbuf_tensor("bufB", [2 * NBB, NT], bf16).ap()
    m2c = nc.alloc_sbuf_tensor("m2c", [N_BINS, NT], f32).ap()
    lnm = nc.alloc_sbuf_tensor("lnm", [N_BINS, NT], f32).ap()
    db = nc.alloc_sbuf_tensor("db", [N_BINS, NT], f32).ap()
    r1u = nc.alloc_sbuf_tensor("r1u", [N_BINS, NI], f32).ap()
    rall = nc.alloc_sbuf_tensor("rall", [N_BINS, NI], f32).ap()
    lnr = nc.alloc_sbuf_tensor("lnr", [N_BINS, NI], f32).ap()
    nlnr = nc.alloc_sbuf_tensor("nlnr", [N_BINS, NI], f32).ap()
    lnwarm = nc.alloc_sbuf_tensor("lnwarm", [1, 3], f32).ap()

    psW = nc.alloc_psum_tensor("psW", [P, NT], f32).ap()
    psA = nc.alloc_psum_tensor("psA", [P, NT], f32).ap()
    psB = nc.alloc_psum_tensor("psB", [2 * NBB, NT], f32).ap()
    psM = nc.alloc_psum_tensor("psM", [N_BINS, NT], f32).ap()

    sactA = wt[:, HDR_SACTA:HDR_SACTA + 4].bitcast(f32)    # [128, 1]
    sactB = wt[:, HDR_SACTB:HDR_SACTB + 4].bitcast(f32)
    SAv = wt[:, HDR_SA:HDR_SA + 2 * SPLIT_BIN].bitcast(bf16)   # [128, 64]
    SBv = wt[:, HDR_SB:HDR_SB + 2 * NBB].bitcast(bf16)         # [128, 20]

    s_x = [nc.alloc_semaphore(f"s_x{k}") for k in range(N_XP)]
    s_w = [nc.alloc_semaphore(f"s_w{s}") for s in range(N_SLABS)]
    s_pe = nc.alloc_semaphore("s_pe")   # 1 psB final, 2 psA final, 3 psM final
    s_a = nc.alloc_semaphore("s_a")     # scalar ACT steps
    s_v = nc.alloc_semaphore("s_v")     # vector steps
    s_g = nc.alloc_semaphore("s_g")     # gpsimd steps
    s_out = nc.alloc_semaphore("s_out")
    s_out2 = nc.alloc_semaphore("s_out2")

    xv = xt.rearrange("p (j r i q) -> p j r i q", j=2, r=4, i=NI)

    def rhs_for(ent):
        (c1, j1), (c2, j2) = ent["s0"], ent["s1"]
        o1, o2 = _xoff(c1, j1), _xoff(c2, j2)
        assert o2 > o1, (c1, j1, c2, j2)
        base = xv[:, j1, c1 % 4, :, (c1 // 4):(c1 // 4) + T]   # [128, NI, T]
        u = base.unsqueeze(1)
        u.ap[1] = [o2 - o1, 2]                                  # [128, 2, NI, T]
        return u

    def lhs_for(e):
        m = SCHEDULE[e]["m"]
        lo = WOFF[e]
        u = wt[:, lo:lo + m].unsqueeze(1)
        u.ap[1] = [_mpad(m), 2]       # [128, 2, m] with 16B-aligned j stride
        return u

    def slab_dma(eng, s):
        lo, hi = _slab_cols(s)
        eng.dma_start(wt[:, lo:hi], w_in[:, lo:hi]).then_inc(s_w[s], 16)

    def x_dma(eng, k):
        lo, hi = XPIECE[k]
        if k < 2:
            src = x_in[0][:, lo:hi]
        else:
            src = x_in[k - 1]
        eng.dma_start(xt[:, lo:hi], src).then_inc(s_x[k], 16)

    outf = out.rearrange("k i t -> k (i t)")

    with nc.Block() as block:

        # DMA choreography: HBM bandwidth is shared round-robin across all
        # in-flight hardware queues, so late big transfers are gated behind
        # the early small ones (ungated they starve the early slabs, which
        # stalls the PE and resets the HAM clock ramp).
        @block.sync
        def _(sync):
            x_dma(sync, 0)
            x_dma(sync, 1)
            sync.wait_ge(s_x[1], 16)
            for k in range(2, N_XP):
                x_dma(sync, k)
            sync.wait_ge(s_a, 6)
            sync.dma_start(outf[:, :2 * T], db[:, :2 * T]).then_inc(s_out, 16)
            if not NOWAIT:
                sync.wait_ge(s_out, 16)

        @block.scalar
        def _(scalar):
            # slabs 0-2 are small and needed first: keep them unthrottled
            # (only x block 0 competes); gate the big tail slabs behind them
            for s in range(min(3, N_SLABS)):
                slab_dma(scalar, s)
            if N_SLABS > 3:
                scalar.wait_ge(s_w[2], 16)
                for s in range(3, N_SLABS):
                    slab_dma(scalar, s)
            # preload ACT tables (Ln + Square + Identity) while DMAs fly
            scalar.activation(lnwarm[:, 0:1], nc.const_aps.tensor(1.0, (1, 1)), Ln)
            scalar.activation(lnwarm[:, 1:2], nc.const_aps.tensor(1.0, (1, 1)),
                              Square)
            scalar.activation(lnwarm[:, 2:3], nc.const_aps.tensor(1.0, (1, 1)),
                              Ident)
            scalar.wait_ge(s_pe, 1)
            scalar.activation(bufB[:], psB[:], Square,
                              scale=sactB[:2 * NBB]).then_inc(s_a)     # 1
            scalar.wait_ge(s_pe, 2)
            scalar.activation(bufA[:], psA[:], Square,
                              scale=sactA[:]).then_inc(s_a)            # 2
            scalar.wait_ge(s_v, 1)
            scalar.activation(lnm[:], m2c[:], Ln).then_inc(s_a)        # 3
            scalar.wait_ge(s_g, 1)
            scalar.activation(lnr[:], rall[:], Ln).then_inc(s_a)       # 4
            scalar.wait_ge(s_v, 3)
            for i in range(2):   # db_i = Identity(DB_SCALE*lnm + (-DB_SCALE*lnr_i))
                scalar.activation(db[:, i * T:(i + 1) * T],
                                  lnm[:, i * T:(i + 1) * T], Ident,
                                  bias=nlnr[:, i:i + 1],
                                  scale=float(DB_SCALE)).then_inc(s_a)  # 5, 6

        @block.vector
        def _(vector):
            vector.wait_ge(s_pe, 3)
            vector.tensor_scalar_max(m2c[:], psM[:],
                                     float(AMIN) ** 2).then_inc(s_v)   # 1
            vector.tensor_reduce(r1u[:], psM.rearrange("p (i f) -> p i f", i=NI),
                                 axis=mybir.AxisListType.X,
                                 op=mybir.AluOpType.max).then_inc(s_v)  # 2
            vector.wait_ge(s_a, 4)
            vector.tensor_scalar_mul(nlnr[:], lnr[:],
                                     -float(DB_SCALE)).then_inc(s_v)   # 3
            for i in range(2, 4):
                vector.tensor_scalar(db[:, i * T:(i + 1) * T],
                                     lnm[:, i * T:(i + 1) * T],
                                     lnr[:, i:i + 1], float(DB_SCALE),
                                     mybir.AluOpType.subtract,
                                     mybir.AluOpType.mult)
            vector.drain().then_inc(s_v)                                # 4

        @block.gpsimd
        def _(gpsimd):
            gpsimd.wait_ge(s_v, 2)
            gpsimd.partition_all_reduce(rall[:], r1u[:], channels=N_BINS,
                                        reduce_op=bass_isa.ReduceOp.max
                                        ).then_inc(s_g)                # 1
            gpsimd.wait_ge(s_v, 4)
            gpsimd.dma_start(outf[:, 2 * T:], db[:, 2 * T:]).then_inc(s_out2, 16)
            if not NOWAIT:
                gpsimd.wait_ge(s_out2, 16)

        @block.tensor
        def _(tensor):
            # HAM warmup on whatever garbage sits in junk's SBUF region:
            # psW is never read and every real accumulation opens with
            # start=True, so the values are irrelevant - starting the PE
            # immediately buys clock-ramp time
            for _ in range(5):
                tensor.matmul(psW[:], lhsT=junk[:, :P], rhs=junk[:, :NT],
                              start=True, stop=True)
            waited = set()

            def need(sem):
                if id(sem) not in waited:
                    tensor.wait_ge(sem, 16)
                    waited.add(id(sem))

            na = sum(1 for e in SCHEDULE if e["kind"] == "A")
            nb = NE - na
            na_seen = nb_seen = 0
            for e, ent in enumerate(SCHEDULE):
                need(s_x[_xblk(*ent["s0"])])
                need(s_x[_xblk(*ent["s1"])])
                need(s_w[_slab_of(e)])
                m = ent["m"]
                ps = psA if ent["kind"] == "A" else psB
                first = (na_seen == 0) if ent["kind"] == "A" else (nb_seen == 0)
                last = (na_seen == na - 1) if ent["kind"] == "A" \
                    else (nb_seen == nb - 1)
                tensor.matmul(ps[:m], lhsT=lhs_for(e), rhs=rhs_for(ent),
                              start=first, stop=last, perf_mode=DR,
                              skip_group_check=True)
                if ent["kind"] == "A":
                    na_seen += 1
                    if na_seen == na:
                        tensor.drain().then_inc(s_pe, 1)       # 2 (B first)
                else:
                    nb_seen += 1
                    if nb_seen == nb:
                        tensor.drain().then_inc(s_pe, 1)       # 1
            # pair-sum matmuls: psM[k] = buf[2k] + buf[2k+1] = re^2 + im^2
            tensor.wait_ge(s_a, 1)
            tensor.matmul(psM[SPLIT_BIN:], lhsT=SBv[:2 * NBB], rhs=bufB[:],
                          start=True, stop=True, skip_group_check=True)
            tensor.wait_ge(s_a, 2)
            tensor.matmul(psM[:SPLIT_BIN], lhsT=SAv[:], rhs=bufA[:],
                          start=True, stop=True, skip_group_check=True)
            tensor.drain().then_inc(s_pe, 1)                   # 3

    nc.compile()
    return nc


_PROGRAM = None


def _get_program():
    global _PROGRAM
    if _PROGRAM is None:
        _PROGRAM = build_program()
    return _PROGRAM


def run(x, **spmd_kwargs):
    """Run on 8 NeuronCores; returns (output [32, 84, 126] f32, results)."""
    nc = _get_program()
    packs = pack_x(x)
    in_maps = [{"x_in": packs[i], "w_in": W_NP} for i in range(N_CORES)]
    res = run_bass_kernel_spmd(nc, in_maps, core_ids=list(range(N_CORES)),
                               **spmd_kwargs)
    out = np.concatenate([res.results[i]["out"].transpose(1, 0, 2)
                          for i in range(N_CORES)], axis=0)
    return np.ascontiguousarray(out.astype(np.float32)), res


def kernel(x):
    return run(x)[0]


# revision 50
# speedup vs baseline: 1.2418x; 1.0413x over previous
"""CQT (constant-Q transform) + amplitude_to_db kernel for Trainium2.

Full-input contract: kernel(x) takes x [32, 64000] f32 and returns
[32, 84, 126] f32, matching:

    frames = pad(x, n_fft//2)[:, t*HOP + n]          # [B, 126, 16384]
    cr/ci  = frames @ Kr.T / Ki.T                    # [B, 84, 126]
    mag    = sqrt(cr^2 + ci^2)
    out    = amplitude_to_db(mag, ref=max per item, amin=1e-5, top_db=80)

Sharding: pure data parallelism - 4 batch items per NeuronCore on 8 cores.

v5: fp8e4 (e4m3) DoubleRow matmuls; one instruction contracts TWO
128-row K-chunk slots (lhsT [128,2,M], rhs [128,2,N]) in the 504 cycles
a single fp16 matmul costs, halving PE time vs fp16.

Sparsity: CQT kernel support halves per octave and is centered, so for
each K-chunk only a PREFIX of bins (lowest ones) is nonzero. Bins are
packed (re,im)-interleaved along psum partitions so each DoubleRow slot
ships only its active prefix (variable stationary width M_e) - about
4x fewer weight bytes than dense 128-wide slots. m2 = re^2+im^2 is then
formed by a tiny pair-summing matmul (S[p,m]=1 iff p//2==m) on the PE.

Precision: single-rounded fp8 pairs give rel_l2 ~1.8e-2 (gate is 2e-2).
The 16 highest-energy (group, chunk) slots instead use error-feedback
"comp" entries: Q0=q(W/2), Q1=q(W-Q0), x0=q(16x), x1=q(32x-x0) and the
two j-slots compute Q0.T x0 + Q1.T x1, halving both quantization noises
where it matters: measured rel_l2 ~1.0e-2.

Per-bin power-of-2 weight scales keep e4m3 in its normal range; the
epilogue unscales for free via ACT per-partition scale operands. GpSimd
runs ONLY the partition all-reduce + one output DMA (element-wise ops
there force a ~7us mid-kernel GpSimd library swap). DMA issues are
gated in need-order: HBM bandwidth is round-robin across in-flight
queues, so ungated late transfers starve the early ones.
"""

import os
import numpy as np
import ml_dtypes

import concourse.bass as bass
import concourse.mybir as mybir
from concourse import bacc
from concourse import bass_isa
from concourse.bass_utils import run_bass_kernel_spmd

# ---- problem constants (hardcoded; must match the reference) ----
SR = 22050
HOP = 512
N_BINS = 84
BPO = 12
FMIN = 32.70319566257483
AMIN = 1e-5
TOP_DB = 80.0
B = 32
N_SAMP = 64000
N_CORES = 8
NI = B // N_CORES            # items per core = 4
T = 1 + N_SAMP // HOP        # 126 frames
NT = NI * T                  # 504
DB_SCALE = 10.0 / np.log(10.0)
P = 128
SPLIT_BIN = 64               # group A: bins [0,64), group B: bins [64,84)
NBB = N_BINS - SPLIT_BIN     # 20
X_SCALE = 16.0
F8 = ml_dtypes.float8_e4m3   # == mybir.dt.float8e4

SCHEME = os.environ.get("CQT_SCHEME", "hybrid")   # hybrid | comp | pairs
# Skip the final output-DMA semaphore waits (teardown then overlaps the
# output DMA). Measured neutral-to-slightly-worse, so off by default.
NOWAIT = os.environ.get("CQT_NOWAIT", "0") == "1"


def _build_cqt_kernels():
    """Same construction as the reference (nnAudio-style direct CQT bank)."""
    Q = 1.0 / (2.0 ** (1.0 / BPO) - 1.0)
    freqs = FMIN * 2.0 ** (np.arange(N_BINS) / BPO)
    lengths = np.ceil(Q * SR / freqs).astype(int)
    n_fft = int(2 ** np.ceil(np.log2(lengths.max())))
    K = np.zeros((N_BINS, n_fft), dtype=np.complex128)
    for k in range(N_BINS):
        L = int(lengths[k])
        t = np.arange(L) - (L - 1) / 2.0
        kern = np.hanning(L) * np.exp(2j * np.pi * freqs[k] * t / SR)
        kern /= np.abs(kern).sum()
        kern /= np.sqrt(L)
        s = (n_fft - L) // 2
        K[k, s:s + L] = kern
    return K.real.astype(np.float32), K.imag.astype(np.float32), n_fft


Kr, Ki, N_FFT = _build_cqt_kernels()
PAD = N_FFT // 2
FW = (N_SAMP + 2 * PAD) // P      # 628
QW = FW // 4                      # 157
XB = 2 * NI * QW                  # 1256, one x-DMA block (2 phases x NI x QW)
assert (N_SAMP + 2 * PAD) % P == 0 and HOP == 4 * P

_NZ = (np.abs(Kr) + np.abs(Ki)) > 0


def _chunk_range(bins):
    nz = _NZ[bins].any(axis=0)
    idx = np.nonzero(nz)[0]
    return int(idx[0]) // P, int(idx[-1]) // P + 1

_A0, _A1 = _chunk_range(range(0, SPLIT_BIN))
_B0, _B1 = _chunk_range(range(SPLIT_BIN, N_BINS))
# Truncate the outermost Hann-tail chunks of group A: the outer 8 chunks
# per side hold ~1e-4 of the low bins' kernel energy, far below the fp8
# quantization noise floor (simulated rel_l2 1.017e-2 vs 1.015e-2).
TRUNC = 8
CHUNKS_A = list(range(_A0 + TRUNC, _A1 - TRUNC))   # 74 chunks
CHUNKS_B = list(range(_B0, _B1))                   # 4 chunks


def _nbins(kind, c):
    """Active-bin count for (group, chunk); active bins are a prefix of
    the group (lowest bins have the widest support)."""
    lo, hi = (0, SPLIT_BIN) if kind == "A" else (SPLIT_BIN, N_BINS)
    act = np.nonzero(_NZ[lo:hi, c * P:(c + 1) * P].any(axis=1))[0]
    assert len(act) > 0 and act[-1] == len(act) - 1, (kind, c, act)
    return int(len(act))

# per-bin power-of-2 scales: peak |w| lands in [80, 160) (e4m3 max = 240)
_wmax = np.maximum(np.abs(Kr).max(axis=1), np.abs(Ki).max(axis=1))
SBIN = 2.0 ** np.floor(np.log2(160.0 / _wmax))

# comp set: top-16 (group, chunk) by filterbank energy (sim rel_l2 1.02e-2)
if SCHEME == "comp":
    COMP = {("A", c) for c in CHUNKS_A} | {("B", c) for c in CHUNKS_B}
elif SCHEME == "pairs":
    COMP = set()
else:
    COMP = ({("B", 63), ("B", 64)} |
            {("A", c) for c in range(57, 71)})


def _xoff(c, j):
    """Column offset of (chunk, j-variant) in the xt free layout
    (j, r, i, q): off = j*2512 + r*628 + i*157 + q."""
    return j * (4 * NI * QW) + (c % 4) * (NI * QW) + (c // 4)


def _xpieces(c, j):
    """x DMA pieces gating slice (c, j): the j0 phase-0 block is split
    in two item-halves on separate queues so the first matmuls can start
    as soon as ~2x40KB has landed in parallel."""
    if j == 0:
        r = c % 4
        if r == 0:
            return (0, 1)
        return (2,) if r == 1 else (3,)
    return (4 + (c % 4) // 2,)


def _xblk(c, j):
    """Max gating piece index, used only for schedule ordering."""
    return max(_xpieces(c, j))


# x DMA pieces as (col_lo, col_hi) of the xt free layout
HQW = NI * QW // 2   # 314: two items' worth of one phase
XPIECE = [(0, HQW), (HQW, 2 * HQW), (NI * QW, 2 * NI * QW), (XB, 2 * XB),
          (2 * XB, 3 * XB), (3 * XB, 4 * XB)]
N_XP = len(XPIECE)


def _build_schedule():
    """Entries: dict(kind, comp, s0, s1, m). Ordering: j0-only pair
    entries by x-block, then comp entries (need j1 blocks); B entries
    lead each segment; a few j0 pairs are held back to the end so the
    last B entry retires well before the last A entry."""
    ents = []
    for kind, chunks in (("A", CHUNKS_A), ("B", CHUNKS_B)):
        kord = 0 if kind == "B" else 1
        comp = [c for c in chunks if (kind, c) in COMP]
        rest = [c for c in chunks if (kind, c) not in COMP]
        for c in comp:
            ents.append(dict(kind=kind, comp=True, s0=(c, 0), s1=(c, 1),
                             m=2 * _nbins(kind, c),
                             key=(_xblk(c, 1), kord, c % 4, c // 4)))
        byphase = {}
        for c in rest:
            byphase.setdefault(c % 4, []).append(c)
        leftovers = []
        for r in sorted(byphase):
            lst = sorted(byphase[r])
            while len(lst) >= 2:
                c1, c2 = lst.pop(0), lst.pop(0)
                ents.append(dict(kind=kind, comp=False, s0=(c1, 0), s1=(c2, 0),
                                 m=2 * max(_nbins(kind, c1), _nbins(kind, c2)),
                                 key=(_xblk(c1, 0), kord, c1 % 4, c1 // 4)))
            leftovers += lst
        leftovers.sort(key=lambda c: _xoff(c, 0))
        while len(leftovers) >= 2:
            c1, c2 = leftovers.pop(0), leftovers.pop(0)
            blk = max(_xblk(c1, 0), _xblk(c2, 0))
            ents.append(dict(kind=kind, comp=False, s0=(c1, 0), s1=(c2, 0),
                             m=2 * max(_nbins(kind, c1), _nbins(kind, c2)),
                             key=(blk, kord, 5, 999)))
        if leftovers:   # odd count: upgrade the last single to a comp entry
            c = leftovers[0]
            ents.append(dict(kind=kind, comp=True, s0=(c, 0), s1=(c, 1),
                             m=2 * _nbins(kind, c),
                             key=(_xblk(c, 1), kord, 5, 999)))
    ents.sort(key=lambda e: e["key"])
    # hold back up to 4 j0-only A pairs to the very end (B-drain slack)
    tail = [e for e in ents if e["kind"] == "A" and not e["comp"]
            and e["key"][0] == 1][-4:]
    for e in tail:
        ents.remove(e)
    ents += tail
    assert ents[-1]["kind"] == "A"
    # the first entry of each group carries start=True, so it must cover
    # the group's full partition range (variable-width entries only touch
    # their prefix partitions)
    for kind, full in (("A", P), ("B", 2 * NBB)):
        first = next(e for e in ents if e["kind"] == kind)
        first["m"] = max(first["m"], full)
    return ents


SCHEDULE = _build_schedule()
NE = len(SCHEDULE)

# header: sactA | sactB scale vectors (fp32), S_A and S_B pair-sum
# matrices (bf16), all bitcast into fp8 columns
HDR_SACTA = 0            # 4 cols  (fp32 [128,1])
HDR_SACTB = 4            # 4 cols
HDR_SA = 8               # 128 cols (bf16 [128, 64])
HDR_SB = 136             # 40 cols  (bf16 [.., 20])
HDR = 176

def _mpad(m):
    """DoubleRow fp8 LDWEIGHTS requires the j-slot stride to be even and
    16B-aligned (s3_lw_dual_fp8_restrictions)."""
    return (m + 15) // 16 * 16


WOFF = []
_off = HDR
for _e in SCHEDULE:
    WOFF.append(_off)
    _off += 2 * _mpad(_e["m"])
W_COLS = _off

# weight slab boundaries: by cumulative bytes (finer early)
_bfr = [0.03, 0.08, 0.18, 0.35, 0.6, 1.0]
SLAB_ENDS = []
prev = 0
for f in _bfr:
    target = HDR + f * (W_COLS - HDR)
    e = prev + 1
    while e < NE and WOFF[e] < target:
        e += 1
    e = min(e, NE)
    if e > prev:
        SLAB_ENDS.append(e)
        prev = e
SLAB_ENDS[-1] = NE
N_SLABS = len(SLAB_ENDS)


def _slab_of(e):
    for s, end in enumerate(SLAB_ENDS):
        if e < end:
            return s
    raise IndexError(e)


def _slab_cols(s):
    lo = 0 if s == 0 else WOFF[SLAB_ENDS[s - 1]] if SLAB_ENDS[s - 1] < NE else W_COLS
    hi = WOFF[SLAB_ENDS[s]] if SLAB_ENDS[s] < NE else W_COLS
    return lo, hi


def _pack_weights():
    q = lambda a: a.astype(F8)
    KrT = (Kr * SBIN[:, None]).T.astype(np.float32)   # [N_FFT, 84]
    KiT = (Ki * SBIN[:, None]).T.astype(np.float32)

    def chunk_w(kind, c, m):
        """Interleaved [128, m] block: col 2k = re(bin k), 2k+1 = im."""
        base = 0 if kind == "A" else SPLIT_BIN
        Wc = np.zeros((P, m), np.float32)
        nb = _nbins(kind, c)
        rows = slice(c * P, (c + 1) * P)
        Wc[:, 0:2 * nb:2] = KrT[rows, base:base + nb]
        Wc[:, 1:2 * nb:2] = KiT[rows, base:base + nb]
        return Wc

    w = np.zeros((P, W_COLS), F8)
    for e, ent in enumerate(SCHEDULE):
        base = WOFF[e]
        m = ent["m"]
        mp = _mpad(m)
        if ent["comp"]:
            Wc = chunk_w(ent["kind"], ent["s0"][0], m)
            Q0 = q(Wc * 0.5)
            Q1 = q(Wc - Q0.astype(np.float32))
            w[:, base:base + m] = Q0
            w[:, base + mp:base + mp + m] = Q1
        else:
            w[:, base:base + m] = q(chunk_w(ent["kind"], ent["s0"][0], m))
            w[:, base + mp:base + mp + m] = q(chunk_w(ent["kind"], ent["s1"][0], m))

    sinv = 1.0 / (SBIN * X_SCALE)
    sactA = np.ones(P, np.float32)
    sactA[:] = sinv[np.arange(P) // 2]                  # psA partition 2k/2k+1
    sactB = np.ones(P, np.float32)
    sactB[:2 * NBB] = sinv[SPLIT_BIN + np.arange(2 * NBB) // 2]
    SA = np.zeros((P, SPLIT_BIN), ml_dtypes.bfloat16)
    SA[np.arange(P), np.arange(P) // 2] = 1.0
    SB = np.zeros((P, NBB), ml_dtypes.bfloat16)
    SB[np.arange(2 * NBB), np.arange(2 * NBB) // 2] = 1.0
    wu8 = w.view(np.uint8)
    wu8[:, HDR_SACTA:HDR_SACTA + 4] = sactA.astype("<f4").view(np.uint8).reshape(P, 4)
    wu8[:, HDR_SACTB:HDR_SACTB + 4] = sactB.astype("<f4").view(np.uint8).reshape(P, 4)
    wu8[:, HDR_SA:HDR_SA + 2 * SPLIT_BIN] = SA.view(np.uint8).reshape(P, -1)
    wu8[:, HDR_SB:HDR_SB + 2 * NBB] = SB.view(np.uint8).reshape(P, -1)
    return w


W_NP = _pack_weights()


def pack_x(x):
    """x [32, 64000] f32 -> per-core fp8 packs [4, 128, 1256].

    SBUF xt free layout (j, r, i, q); DMA block k = 2j + r//2.
    x0 = q(16 x); x1 = q(32 x - x0)."""
    xs = np.asarray(x, dtype=np.float32)
    xp = np.pad(xs, ((0, 0), (PAD, PAD)))                 # [32, 80384]
    x0 = (xp * X_SCALE).astype(F8)
    x1 = (2.0 * X_SCALE * xp - x0.astype(np.float32)).astype(F8)
    # sample n = 512 q + 128 r + p  ->  [j, bi, q, r, p]
    X = np.stack([x0, x1]).reshape(2, B, QW, 4, P)
    X = X.reshape(2, B, QW, 2, 2, P)                      # [j, bi, q, rblk, rib, p]
    packs = []
    for core in range(N_CORES):
        blk = X[:, core * NI:(core + 1) * NI]             # [j, i, q, rblk, rib, p]
        arr = blk.transpose(0, 3, 5, 4, 1, 2)             # [j, rblk, p, rib, i, q]
        packs.append(np.ascontiguousarray(arr.reshape(4, P, XB)))
    return packs


def build_program():
    nc = bacc.Bacc("TRN2", target_bir_lowering=False, debug=False,
                   enable_asserts=True)
    f8 = mybir.dt.float8e4
    bf16 = mybir.dt.bfloat16
    f32 = mybir.dt.float32
    DR = mybir.MatmulPerfMode.DoubleRow
    Ln = mybir.ActivationFunctionType.Ln
    Square = mybir.ActivationFunctionType.Square
    Ident = mybir.ActivationFunctionType.Identity

    x_in = nc.dram_tensor("x_in", [4, P, XB], f8, kind="ExternalInput").ap()
    w_in = nc.dram_tensor("w_in", [P, W_COLS], f8, kind="ExternalInput").ap()
    out = nc.dram_tensor("out", [N_BINS, NI, T], f32, kind="ExternalOutput").ap()

    xt = nc.alloc_sbuf_tensor("xt", [P, 4 * XB], f8).ap()
    wt = nc.alloc_sbuf_tensor("wt", [P, W_COLS], f8).ap()
    junk = nc.alloc_sbuf_tensor("junk", [P, 512], f8).ap()
    bufA = nc.alloc_sbuf_tensor("bufA", [P, NT], bf16).ap()
    bufB = nc.alloc_sbuf_tensor("bufB", [2 * NBB, NT], bf16).ap()
    m2c = nc.alloc_sbuf_tensor("m2c", [N_BINS, NT], f32).ap()
    lnm = nc.alloc_sbuf_tensor("lnm", [N_BINS, NT], f32).ap()
    db = nc.alloc_sbuf_tensor("db", [N_BINS, NT], f32).ap()
    r1u = nc.alloc_sbuf_tensor("r1u", [N_BINS, NI], f32).ap()
    rall = nc.alloc_sbuf_tensor("rall", [N_BINS, NI], f32).ap()
    lnr = nc.alloc_sbuf_tensor("lnr", [N_BINS, NI], f32).ap()
    nlnr = nc.alloc_sbuf_tensor("nlnr", [N_BINS, NI], f32).ap()
    lnwarm = nc.alloc_sbuf_tensor("lnwarm", [1, 3], f32).ap()

    psW = nc.alloc_psum_tensor("psW", [P, NT], f32).ap()
    psA = nc.alloc_psum_tensor("psA", [P, NT], f32).ap()
    psB = nc.alloc_psum_tensor("psB", [2 * NBB, NT], f32).ap()
    psM = nc.alloc_psum_tensor("psM", [N_BINS, NT], f32).ap()

    sactA = wt[:, HDR_SACTA:HDR_SACTA + 4].bitcast(f32)    # [128, 1]
    sactB = wt[:, HDR_SACTB:HDR_SACTB + 4].bitcast(f32)
    SAv = wt[:, HDR_SA:HDR_SA + 2 * SPLIT_BIN].bitcast(bf16)   # [128, 64]
    SBv = wt[:, HDR_SB:HDR_SB + 2 * NBB].bitcast(bf16)         # [128, 20]

    s_x = [nc.alloc_semaphore(f"s_x{k}") for k in range(N_XP)]
    s_w = [nc.alloc_semaphore(f"s_w{s}") for s in range(N_SLABS)]
    s_pe = nc.alloc_semaphore("s_pe")   # 1 psB final, 2 psA final, 3 psM final
    s_a = nc.alloc_semaphore("s_a")     # scalar ACT steps
    s_v = nc.alloc_semaphore("s_v")     # vector steps
    s_g = nc.alloc_semaphore("s_g")     # gpsimd steps
    s_out = nc.alloc_semaphore("s_out")
    s_out2 = nc.alloc_semaphore("s_out2")

    xv = xt.rearrange("p (j r i q) -> p j r i q", j=2, r=4, i=NI)

    def rhs_for(ent):
        (c1, j1), (c2, j2) = ent["s0"], ent["s1"]
        o1, o2 = _xoff(c1, j1), _xoff(c2, j2)
        assert o2 > o1, (c1, j1, c2, j2)
        base = xv[:, j1, c1 % 4, :, (c1 // 4):(c1 // 4) + T]   # [128, NI, T]
        u = base.unsqueeze(1)
        u.ap[1] = [o2 - o1, 2]                                  # [128, 2, NI, T]
        return u

    def lhs_for(e):
        m = SCHEDULE[e]["m"]
        lo = WOFF[e]
        u = wt[:, lo:lo + m].unsqueeze(1)
        u.ap[1] = [_mpad(m), 2]       # [128, 2, m] with 16B-aligned j stride
        return u

    def slab_dma(eng, s):
        lo, hi = _slab_cols(s)
        eng.dma_start(wt[:, lo:hi], w_in[:, lo:hi]).then_inc(s_w[s], 16)

    def x_dma(eng, k):
        lo, hi = XPIECE[k]
        if k < 3:
            src = x_in[0][:, lo:hi]
        else:
            src = x_in[k - 2]
        eng.dma_start(xt[:, lo:hi], src).then_inc(s_x[k], 16)

    outf = out.rearrange("k i t -> k (i t)")

    with nc.Block() as block:

        # DMA choreography: HBM bandwidth is shared round-robin across all
        # in-flight hardware queues, so late big transfers are gated behind
        # the early small ones (ungated they starve the early slabs, which
        # stalls the PE and resets the HAM clock ramp).
        @block.sync
        def _(sync):
            x_dma(sync, 0)      # (j0, r0) items 0-1; items 2-3 on gpsimd
            x_dma(sync, 2)      # (j0, r1)
            sync.wait_ge(s_x[2], 16)
            for k in range(3, N_XP):
                x_dma(sync, k)
            sync.wait_ge(s_a, 6)
            sync.dma_start(outf[:, :2 * T], db[:, :2 * T]).then_inc(s_out, 16)
            if not NOWAIT:
                sync.wait_ge(s_out, 16)

        @block.scalar
        def _(scalar):
            # slabs 0-2 are small and needed first: keep them unthrottled
            # (only x block 0 competes); gate the big tail slabs behind them
            for s in range(min(3, N_SLABS)):
                slab_dma(scalar, s)
            if N_SLABS > 3:
                scalar.wait_ge(s_w[2], 16)
                for s in range(3, N_SLABS):
                    slab_dma(scalar, s)
            # preload ACT tables (Ln + Square + Identity) while DMAs fly
            scalar.activation(lnwarm[:, 0:1], nc.const_aps.tensor(1.0, (1, 1)), Ln)
            scalar.activation(lnwarm[:, 1:2], nc.const_aps.tensor(1.0, (1, 1)),
                              Square)
            scalar.activation(lnwarm[:, 2:3], nc.const_aps.tensor(1.0, (1, 1)),
                              Ident)
            scalar.wait_ge(s_pe, 1)
            scalar.activation(bufB[:], psB[:], Square,
                              scale=sactB[:2 * NBB]).then_inc(s_a)     # 1
            scalar.wait_ge(s_pe, 2)
            scalar.activation(bufA[:], psA[:], Square,
                              scale=sactA[:]).then_inc(s_a)            # 2
            scalar.wait_ge(s_v, 1)
            scalar.activation(lnm[:], m2c[:], Ln).then_inc(s_a)        # 3
            scalar.wait_ge(s_g, 1)
            scalar.activation(lnr[:], rall[:], Ln).then_inc(s_a)       # 4
            scalar.wait_ge(s_v, 3)
            for i in range(2):   # db_i = Identity(DB_SCALE*lnm + (-DB_SCALE*lnr_i))
                scalar.activation(db[:, i * T:(i + 1) * T],
                                  lnm[:, i * T:(i + 1) * T], Ident,
                                  bias=nlnr[:, i:i + 1],
                                  scale=float(DB_SCALE)).then_inc(s_a)  # 5, 6

        @block.vector
        def _(vector):
            vector.wait_ge(s_pe, 3)
            vector.tensor_scalar_max(m2c[:], psM[:],
                                     float(AMIN) ** 2).then_inc(s_v)   # 1
            vector.tensor_reduce(r1u[:], psM.rearrange("p (i f) -> p i f", i=NI),
                                 axis=mybir.AxisListType.X,
                                 op=mybir.AluOpType.max).then_inc(s_v)  # 2
            vector.wait_ge(s_a, 4)
            vector.tensor_scalar_mul(nlnr[:], lnr[:],
                                     -float(DB_SCALE)).then_inc(s_v)   # 3
            for i in range(2, 4):
                vector.tensor_scalar(db[:, i * T:(i + 1) * T],
                                     lnm[:, i * T:(i + 1) * T],
                                     lnr[:, i:i + 1], float(DB_SCALE),
                                     mybir.AluOpType.subtract,
                                     mybir.AluOpType.mult)
            vector.drain().then_inc(s_v)                                # 4

        @block.gpsimd
        def _(gpsimd):
            x_dma(gpsimd, 1)    # (j0, r0) items 2-3
            gpsimd.wait_ge(s_v, 2)
            gpsimd.partition_all_reduce(rall[:], r1u[:], channels=N_BINS,
                                        reduce_op=bass_isa.ReduceOp.max
                                        ).then_inc(s_g)                # 1
            gpsimd.wait_ge(s_v, 4)
            gpsimd.dma_start(outf[:, 2 * T:], db[:, 2 * T:]).then_inc(s_out2, 16)
            if not NOWAIT:
                gpsimd.wait_ge(s_out2, 16)

        @block.tensor
        def _(tensor):
            # HAM warmup on whatever garbage sits in junk's SBUF region:
            # psW is never read and every real accumulation opens with
            # start=True, so the values are irrelevant - starting the PE
            # immediately buys clock-ramp time
            for _ in range(5):
                tensor.matmul(psW[:], lhsT=junk[:, :P], rhs=junk[:, :NT],
                              start=True, stop=True)
            waited = set()

            def need(sem):
                if id(sem) not in waited:
                    tensor.wait_ge(sem, 16)
                    waited.add(id(sem))

            na = sum(1 for e in SCHEDULE if e["kind"] == "A")
            nb = NE - na
            na_seen = nb_seen = 0
            for e, ent in enumerate(SCHEDULE):
                for sl in (ent["s0"], ent["s1"]):
                    for k in _xpieces(*sl):
                        need(s_x[k])
                need(s_w[_slab_of(e)])
                m = ent["m"]
                ps = psA if ent["kind"] == "A" else psB
                first = (na_seen == 0) if ent["kind"] == "A" else (nb_seen == 0)
                last = (na_seen == na - 1) if ent["kind"] == "A" \
                    else (nb_seen == nb - 1)
                tensor.matmul(ps[:m], lhsT=lhs_for(e), rhs=rhs_for(ent),
                              start=first, stop=last, perf_mode=DR,
                              skip_group_check=True)
                if ent["kind"] == "A":
                    na_seen += 1
                    if na_seen == na:
                        tensor.drain().then_inc(s_pe, 1)       # 2 (B first)
                else:
                    nb_seen += 1
                    if nb_seen == nb:
                        tensor.drain().then_inc(s_pe, 1)       # 1
            # pair-sum matmuls: psM[k] = buf[2k] + buf[2k+1] = re^2 + im^2
            tensor.wait_ge(s_a, 1)
            tensor.matmul(psM[SPLIT_BIN:], lhsT=SBv[:2 * NBB], rhs=bufB[:],
                          start=True, stop=True, skip_group_check=True)
            tensor.wait_ge(s_a, 2)
            tensor.matmul(psM[:SPLIT_BIN], lhsT=SAv[:], rhs=bufA[:],
                          start=True, stop=True, skip_group_check=True)
            tensor.drain().then_inc(s_pe, 1)                   # 3

    nc.compile()
    return nc


_PROGRAM = None


def _get_program():
    global _PROGRAM
    if _PROGRAM is None:
        _PROGRAM = build_program()
    return _PROGRAM


def run(x, **spmd_kwargs):
    """Run on 8 NeuronCores; returns (output [32, 84, 126] f32, results)."""
    nc = _get_program()
    packs = pack_x(x)
    in_maps = [{"x_in": packs[i], "w_in": W_NP} for i in range(N_CORES)]
    res = run_bass_kernel_spmd(nc, in_maps, core_ids=list(range(N_CORES)),
                               **spmd_kwargs)
    out = np.concatenate([res.results[i]["out"].transpose(1, 0, 2)
                          for i in range(N_CORES)], axis=0)
    return np.ascontiguousarray(out.astype(np.float32)), res


def kernel(x):
    return run(x)[0]


# revision 51
# speedup vs baseline: 1.2431x; 1.0010x over previous
"""CQT (constant-Q transform) + amplitude_to_db kernel for Trainium2.

Full-input contract: kernel(x) takes x [32, 64000] f32 and returns
[32, 84, 126] f32, matching:

    frames = pad(x, n_fft//2)[:, t*HOP + n]          # [B, 126, 16384]
    cr/ci  = frames @ Kr.T / Ki.T                    # [B, 84, 126]
    mag    = sqrt(cr^2 + ci^2)
    out    = amplitude_to_db(mag, ref=max per item, amin=1e-5, top_db=80)

Sharding: pure data parallelism - 4 batch items per NeuronCore on 8 cores.

v5: fp8e4 (e4m3) DoubleRow matmuls; one instruction contracts TWO
128-row K-chunk slots (lhsT [128,2,M], rhs [128,2,N]) in the 504 cycles
a single fp16 matmul costs, halving PE time vs fp16.

Sparsity: CQT kernel support halves per octave and is centered, so for
each K-chunk only a PREFIX of bins (lowest ones) is nonzero. Bins are
packed (re,im)-interleaved along psum partitions so each DoubleRow slot
ships only its active prefix (variable stationary width M_e) - about
4x fewer weight bytes than dense 128-wide slots. m2 = re^2+im^2 is then
formed by a tiny pair-summing matmul (S[p,m]=1 iff p//2==m) on the PE.

Precision: single-rounded fp8 pairs give rel_l2 ~1.8e-2 (gate is 2e-2).
The 16 highest-energy (group, chunk) slots instead use error-feedback
"comp" entries: Q0=q(W/2), Q1=q(W-Q0), x0=q(16x), x1=q(32x-x0) and the
two j-slots compute Q0.T x0 + Q1.T x1, halving both quantization noises
where it matters: measured rel_l2 ~1.0e-2.

Per-bin power-of-2 weight scales keep e4m3 in its normal range; the
epilogue unscales for free via ACT per-partition scale operands. GpSimd
runs ONLY the partition all-reduce + one output DMA (element-wise ops
there force a ~7us mid-kernel GpSimd library swap). DMA issues are
gated in need-order: HBM bandwidth is round-robin across in-flight
queues, so ungated late transfers starve the early ones.
"""

import os
import numpy as np
import ml_dtypes

import concourse.bass as bass
import concourse.mybir as mybir
from concourse import bacc
from concourse import bass_isa
from concourse.bass_utils import run_bass_kernel_spmd

# ---- problem constants (hardcoded; must match the reference) ----
SR = 22050
HOP = 512
N_BINS = 84
BPO = 12
FMIN = 32.70319566257483
AMIN = 1e-5
TOP_DB = 80.0
B = 32
N_SAMP = 64000
N_CORES = 8
NI = B // N_CORES            # items per core = 4
T = 1 + N_SAMP // HOP        # 126 frames
NT = NI * T                  # 504
DB_SCALE = 10.0 / np.log(10.0)
P = 128
SPLIT_BIN = 64               # group A: bins [0,64), group B: bins [64,84)
NBB = N_BINS - SPLIT_BIN     # 20
X_SCALE = 16.0
F8 = ml_dtypes.float8_e4m3   # == mybir.dt.float8e4

SCHEME = os.environ.get("CQT_SCHEME", "hybrid")   # hybrid | comp | pairs
# Skip the final output-DMA semaphore waits (teardown then overlaps the
# output DMA). Measured neutral-to-slightly-worse, so off by default.
NOWAIT = os.environ.get("CQT_NOWAIT", "0") == "1"


def _build_cqt_kernels():
    """Same construction as the reference (nnAudio-style direct CQT bank)."""
    Q = 1.0 / (2.0 ** (1.0 / BPO) - 1.0)
    freqs = FMIN * 2.0 ** (np.arange(N_BINS) / BPO)
    lengths = np.ceil(Q * SR / freqs).astype(int)
    n_fft = int(2 ** np.ceil(np.log2(lengths.max())))
    K = np.zeros((N_BINS, n_fft), dtype=np.complex128)
    for k in range(N_BINS):
        L = int(lengths[k])
        t = np.arange(L) - (L - 1) / 2.0
        kern = np.hanning(L) * np.exp(2j * np.pi * freqs[k] * t / SR)
        kern /= np.abs(kern).sum()
        kern /= np.sqrt(L)
        s = (n_fft - L) // 2
        K[k, s:s + L] = kern
    return K.real.astype(np.float32), K.imag.astype(np.float32), n_fft


Kr, Ki, N_FFT = _build_cqt_kernels()
PAD = N_FFT // 2
FW = (N_SAMP + 2 * PAD) // P      # 628
QW = FW // 4                      # 157
XB = 2 * NI * QW                  # 1256, one x-DMA block (2 phases x NI x QW)
assert (N_SAMP + 2 * PAD) % P == 0 and HOP == 4 * P

_NZ = (np.abs(Kr) + np.abs(Ki)) > 0


def _chunk_range(bins):
    nz = _NZ[bins].any(axis=0)
    idx = np.nonzero(nz)[0]
    return int(idx[0]) // P, int(idx[-1]) // P + 1

_A0, _A1 = _chunk_range(range(0, SPLIT_BIN))
_B0, _B1 = _chunk_range(range(SPLIT_BIN, N_BINS))
# Truncate the outermost Hann-tail chunks of group A: the outer 8 chunks
# per side hold ~1e-4 of the low bins' kernel energy, far below the fp8
# quantization noise floor (simulated rel_l2 1.017e-2 vs 1.015e-2).
TRUNC = 8
CHUNKS_A = list(range(_A0 + TRUNC, _A1 - TRUNC))   # 74 chunks
CHUNKS_B = list(range(_B0, _B1))                   # 4 chunks


def _nbins(kind, c):
    """Active-bin count for (group, chunk); active bins are a prefix of
    the group (lowest bins have the widest support)."""
    lo, hi = (0, SPLIT_BIN) if kind == "A" else (SPLIT_BIN, N_BINS)
    act = np.nonzero(_NZ[lo:hi, c * P:(c + 1) * P].any(axis=1))[0]
    assert len(act) > 0 and act[-1] == len(act) - 1, (kind, c, act)
    return int(len(act))

# per-bin power-of-2 scales: peak |w| lands in [80, 160) (e4m3 max = 240)
_wmax = np.maximum(np.abs(Kr).max(axis=1), np.abs(Ki).max(axis=1))
SBIN = 2.0 ** np.floor(np.log2(160.0 / _wmax))

# comp set: top-16 (group, chunk) by filterbank energy (sim rel_l2 1.02e-2)
if SCHEME == "comp":
    COMP = {("A", c) for c in CHUNKS_A} | {("B", c) for c in CHUNKS_B}
elif SCHEME == "pairs":
    COMP = set()
else:
    COMP = ({("B", 63), ("B", 64)} |
            {("A", c) for c in range(57, 71)})


def _xoff(c, j):
    """Column offset of (chunk, j-variant) in the xt free layout
    (j, r, i, q): off = j*2512 + r*628 + i*157 + q."""
    return j * (4 * NI * QW) + (c % 4) * (NI * QW) + (c // 4)


def _xpieces(c, j):
    """x DMA pieces gating slice (c, j): the j0 phase-0 block is split
    in two item-halves on separate queues so the first matmuls can start
    as soon as ~2x40KB has landed in parallel."""
    if j == 0:
        r = c % 4
        if r == 0:
            return (0, 1)
        return (2,) if r == 1 else (3,)
    return (4 + (c % 4) // 2,)


def _xblk(c, j):
    """Max gating piece index, used only for schedule ordering."""
    return max(_xpieces(c, j))


# x DMA pieces as (col_lo, col_hi) of the xt free layout
HQW = NI * QW // 2   # 314: two items' worth of one phase
XPIECE = [(0, HQW), (HQW, 2 * HQW), (NI * QW, 2 * NI * QW), (XB, 2 * XB),
          (2 * XB, 3 * XB), (3 * XB, 4 * XB)]
N_XP = len(XPIECE)


def _build_schedule():
    """Entries: dict(kind, comp, s0, s1, m). Ordering: j0-only pair
    entries by x-block, then comp entries (need j1 blocks); B entries
    lead each segment; a few j0 pairs are held back to the end so the
    last B entry retires well before the last A entry."""
    ents = []
    for kind, chunks in (("A", CHUNKS_A), ("B", CHUNKS_B)):
        kord = 0 if kind == "B" else 1
        comp = [c for c in chunks if (kind, c) in COMP]
        rest = [c for c in chunks if (kind, c) not in COMP]
        for c in comp:
            ents.append(dict(kind=kind, comp=True, s0=(c, 0), s1=(c, 1),
                             m=2 * _nbins(kind, c),
                             key=(_xblk(c, 1), kord, c % 4, c // 4)))
        byphase = {}
        for c in rest:
            byphase.setdefault(c % 4, []).append(c)
        leftovers = []
        for r in sorted(byphase):
            lst = sorted(byphase[r])
            while len(lst) >= 2:
                c1, c2 = lst.pop(0), lst.pop(0)
                ents.append(dict(kind=kind, comp=False, s0=(c1, 0), s1=(c2, 0),
                                 m=2 * max(_nbins(kind, c1), _nbins(kind, c2)),
                                 key=(_xblk(c1, 0), kord, c1 % 4, c1 // 4)))
            leftovers += lst
        leftovers.sort(key=lambda c: _xoff(c, 0))
        while len(leftovers) >= 2:
            c1, c2 = leftovers.pop(0), leftovers.pop(0)
            blk = max(_xblk(c1, 0), _xblk(c2, 0))
            ents.append(dict(kind=kind, comp=False, s0=(c1, 0), s1=(c2, 0),
                             m=2 * max(_nbins(kind, c1), _nbins(kind, c2)),
                             key=(blk, kord, 5, 999)))
        if leftovers:   # odd count: upgrade the last single to a comp entry
            c = leftovers[0]
            ents.append(dict(kind=kind, comp=True, s0=(c, 0), s1=(c, 1),
                             m=2 * _nbins(kind, c),
                             key=(_xblk(c, 1), kord, 5, 999)))
    ents.sort(key=lambda e: e["key"])
    # hold back up to 4 j0-only A pairs to the very end (B-drain slack)
    tail = [e for e in ents if e["kind"] == "A" and not e["comp"]
            and e["key"][0] == 1][-4:]
    for e in tail:
        ents.remove(e)
    ents += tail
    assert ents[-1]["kind"] == "A"
    # the first entry of each group carries start=True, so it must cover
    # the group's full partition range (variable-width entries only touch
    # their prefix partitions)
    for kind, full in (("A", P), ("B", 2 * NBB)):
        first = next(e for e in ents if e["kind"] == kind)
        first["m"] = max(first["m"], full)
    return ents


SCHEDULE = _build_schedule()
NE = len(SCHEDULE)

# header: sactA | sactB scale vectors (fp32), S_A and S_B pair-sum
# matrices (bf16), all bitcast into fp8 columns
HDR_SACTA = 0            # 4 cols  (fp32 [128,1])
HDR_SACTB = 4            # 4 cols
HDR_SA = 8               # 128 cols (bf16 [128, 64])
HDR_SB = 136             # 40 cols  (bf16 [.., 20])
HDR = 176

def _mpad(m):
    """DoubleRow fp8 LDWEIGHTS requires the j-slot stride to be even and
    16B-aligned (s3_lw_dual_fp8_restrictions)."""
    return (m + 15) // 16 * 16


WOFF = []
_off = HDR
for _e in SCHEDULE:
    WOFF.append(_off)
    _off += 2 * _mpad(_e["m"])
W_COLS = _off

# weight slab boundaries: by cumulative bytes (finer early)
_bfr = [0.03, 0.08, 0.18, 0.35, 0.6, 1.0]
SLAB_ENDS = []
prev = 0
for f in _bfr:
    target = HDR + f * (W_COLS - HDR)
    e = prev + 1
    while e < NE and WOFF[e] < target:
        e += 1
    e = min(e, NE)
    if e > prev:
        SLAB_ENDS.append(e)
        prev = e
SLAB_ENDS[-1] = NE
N_SLABS = len(SLAB_ENDS)


def _slab_of(e):
    for s, end in enumerate(SLAB_ENDS):
        if e < end:
            return s
    raise IndexError(e)


def _slab_cols(s):
    lo = 0 if s == 0 else WOFF[SLAB_ENDS[s - 1]] if SLAB_ENDS[s - 1] < NE else W_COLS
    hi = WOFF[SLAB_ENDS[s]] if SLAB_ENDS[s] < NE else W_COLS
    return lo, hi


def _pack_weights():
    q = lambda a: a.astype(F8)
    KrT = (Kr * SBIN[:, None]).T.astype(np.float32)   # [N_FFT, 84]
    KiT = (Ki * SBIN[:, None]).T.astype(np.float32)

    def chunk_w(kind, c, m):
        """Interleaved [128, m] block: col 2k = re(bin k), 2k+1 = im."""
        base = 0 if kind == "A" else SPLIT_BIN
        Wc = np.zeros((P, m), np.float32)
        nb = _nbins(kind, c)
        rows = slice(c * P, (c + 1) * P)
        Wc[:, 0:2 * nb:2] = KrT[rows, base:base + nb]
        Wc[:, 1:2 * nb:2] = KiT[rows, base:base + nb]
        return Wc

    w = np.zeros((P, W_COLS), F8)
    for e, ent in enumerate(SCHEDULE):
        base = WOFF[e]
        m = ent["m"]
        mp = _mpad(m)
        if ent["comp"]:
            Wc = chunk_w(ent["kind"], ent["s0"][0], m)
            Q0 = q(Wc * 0.5)
            Q1 = q(Wc - Q0.astype(np.float32))
            w[:, base:base + m] = Q0
            w[:, base + mp:base + mp + m] = Q1
        else:
            w[:, base:base + m] = q(chunk_w(ent["kind"], ent["s0"][0], m))
            w[:, base + mp:base + mp + m] = q(chunk_w(ent["kind"], ent["s1"][0], m))

    sinv = 1.0 / (SBIN * X_SCALE)
    sactA = np.ones(P, np.float32)
    sactA[:] = sinv[np.arange(P) // 2]                  # psA partition 2k/2k+1
    sactB = np.ones(P, np.float32)
    sactB[:2 * NBB] = sinv[SPLIT_BIN + np.arange(2 * NBB) // 2]
    SA = np.zeros((P, SPLIT_BIN), ml_dtypes.bfloat16)
    SA[np.arange(P), np.arange(P) // 2] = 1.0
    SB = np.zeros((P, NBB), ml_dtypes.bfloat16)
    SB[np.arange(2 * NBB), np.arange(2 * NBB) // 2] = 1.0
    wu8 = w.view(np.uint8)
    wu8[:, HDR_SACTA:HDR_SACTA + 4] = sactA.astype("<f4").view(np.uint8).reshape(P, 4)
    wu8[:, HDR_SACTB:HDR_SACTB + 4] = sactB.astype("<f4").view(np.uint8).reshape(P, 4)
    wu8[:, HDR_SA:HDR_SA + 2 * SPLIT_BIN] = SA.view(np.uint8).reshape(P, -1)
    wu8[:, HDR_SB:HDR_SB + 2 * NBB] = SB.view(np.uint8).reshape(P, -1)
    return w


W_NP = _pack_weights()


def pack_x(x):
    """x [32, 64000] f32 -> per-core fp8 packs [4, 128, 1256].

    SBUF xt free layout (j, r, i, q); DMA block k = 2j + r//2.
    x0 = q(16 x); x1 = q(32 x - x0)."""
    xs = np.asarray(x, dtype=np.float32)
    xp = np.pad(xs, ((0, 0), (PAD, PAD)))                 # [32, 80384]
    x0 = (xp * X_SCALE).astype(F8)
    x1 = (2.0 * X_SCALE * xp - x0.astype(np.float32)).astype(F8)
    # sample n = 512 q + 128 r + p  ->  [j, bi, q, r, p]
    X = np.stack([x0, x1]).reshape(2, B, QW, 4, P)
    X = X.reshape(2, B, QW, 2, 2, P)                      # [j, bi, q, rblk, rib, p]
    packs = []
    for core in range(N_CORES):
        blk = X[:, core * NI:(core + 1) * NI]             # [j, i, q, rblk, rib, p]
        arr = blk.transpose(0, 3, 5, 4, 1, 2)             # [j, rblk, p, rib, i, q]
        packs.append(np.ascontiguousarray(arr.reshape(4, P, XB)))
    return packs


def build_program():
    nc = bacc.Bacc("TRN2", target_bir_lowering=False, debug=False,
                   enable_asserts=True)
    f8 = mybir.dt.float8e4
    bf16 = mybir.dt.bfloat16
    f32 = mybir.dt.float32
    DR = mybir.MatmulPerfMode.DoubleRow
    Ln = mybir.ActivationFunctionType.Ln
    Square = mybir.ActivationFunctionType.Square
    Ident = mybir.ActivationFunctionType.Identity

    x_in = nc.dram_tensor("x_in", [4, P, XB], f8, kind="ExternalInput").ap()
    w_in = nc.dram_tensor("w_in", [P, W_COLS], f8, kind="ExternalInput").ap()
    out = nc.dram_tensor("out", [N_BINS, NI, T], f32, kind="ExternalOutput").ap()

    xt = nc.alloc_sbuf_tensor("xt", [P, 4 * XB], f8).ap()
    wt = nc.alloc_sbuf_tensor("wt", [P, W_COLS], f8).ap()
    junk = nc.alloc_sbuf_tensor("junk", [P, 512], f8).ap()
    bufA = nc.alloc_sbuf_tensor("bufA", [P, NT], bf16).ap()
    bufB = nc.alloc_sbuf_tensor("bufB", [2 * NBB, NT], bf16).ap()
    m2c = nc.alloc_sbuf_tensor("m2c", [N_BINS, NT], f32).ap()
    lnm = nc.alloc_sbuf_tensor("lnm", [N_BINS, NT], f32).ap()
    db = nc.alloc_sbuf_tensor("db", [N_BINS, NT], f32).ap()
    r1u = nc.alloc_sbuf_tensor("r1u", [N_BINS, NI], f32).ap()
    rall = nc.alloc_sbuf_tensor("rall", [N_BINS, NI], f32).ap()
    lnr = nc.alloc_sbuf_tensor("lnr", [N_BINS, NI], f32).ap()
    nlnr = nc.alloc_sbuf_tensor("nlnr", [N_BINS, NI], f32).ap()
    lnwarm = nc.alloc_sbuf_tensor("lnwarm", [1, 3], f32).ap()

    psW = nc.alloc_psum_tensor("psW", [P, NT], f32).ap()
    psA = nc.alloc_psum_tensor("psA", [P, NT], f32).ap()
    psB = nc.alloc_psum_tensor("psB", [2 * NBB, NT], f32).ap()
    psM = nc.alloc_psum_tensor("psM", [N_BINS, NT], f32).ap()

    sactA = wt[:, HDR_SACTA:HDR_SACTA + 4].bitcast(f32)    # [128, 1]
    sactB = wt[:, HDR_SACTB:HDR_SACTB + 4].bitcast(f32)
    SAv = wt[:, HDR_SA:HDR_SA + 2 * SPLIT_BIN].bitcast(bf16)   # [128, 64]
    SBv = wt[:, HDR_SB:HDR_SB + 2 * NBB].bitcast(bf16)         # [128, 20]

    s_x = [nc.alloc_semaphore(f"s_x{k}") for k in range(N_XP)]
    s_w = [nc.alloc_semaphore(f"s_w{s}") for s in range(N_SLABS)]
    s_pe = nc.alloc_semaphore("s_pe")   # 1 psB final, 2 psA final, 3 psM final
    s_a = nc.alloc_semaphore("s_a")     # scalar ACT steps
    s_v = nc.alloc_semaphore("s_v")     # vector steps
    s_g = nc.alloc_semaphore("s_g")     # gpsimd steps
    s_out = nc.alloc_semaphore("s_out")
    s_out2 = nc.alloc_semaphore("s_out2")

    xv = xt.rearrange("p (j r i q) -> p j r i q", j=2, r=4, i=NI)

    def rhs_for(ent):
        (c1, j1), (c2, j2) = ent["s0"], ent["s1"]
        o1, o2 = _xoff(c1, j1), _xoff(c2, j2)
        assert o2 > o1, (c1, j1, c2, j2)
        base = xv[:, j1, c1 % 4, :, (c1 // 4):(c1 // 4) + T]   # [128, NI, T]
        u = base.unsqueeze(1)
        u.ap[1] = [o2 - o1, 2]                                  # [128, 2, NI, T]
        return u

    def lhs_for(e):
        m = SCHEDULE[e]["m"]
        lo = WOFF[e]
        u = wt[:, lo:lo + m].unsqueeze(1)
        u.ap[1] = [_mpad(m), 2]       # [128, 2, m] with 16B-aligned j stride
        return u

    def slab_dma(eng, s):
        lo, hi = _slab_cols(s)
        eng.dma_start(wt[:, lo:hi], w_in[:, lo:hi]).then_inc(s_w[s], 16)

    def x_dma(eng, k):
        lo, hi = XPIECE[k]
        if k < 3:
            src = x_in[0][:, lo:hi]
        else:
            src = x_in[k - 2]
        eng.dma_start(xt[:, lo:hi], src).then_inc(s_x[k], 16)

    outf = out.rearrange("k i t -> k (i t)")

    with nc.Block() as block:

        # DMA choreography: HBM bandwidth is shared round-robin across all
        # in-flight hardware queues, so late big transfers are gated behind
        # the early small ones (ungated they starve the early slabs, which
        # stalls the PE and resets the HAM clock ramp).
        @block.sync
        def _(sync):
            x_dma(sync, 0)      # (j0, r0) items 0-1; items 2-3 on gpsimd
            x_dma(sync, 2)      # (j0, r1)
            sync.wait_ge(s_x[2], 16)
            for k in range(3, N_XP):
                x_dma(sync, k)
            sync.wait_ge(s_a, 6)
            sync.dma_start(outf[:, :2 * T], db[:, :2 * T]).then_inc(s_out, 16)
            if not NOWAIT:
                sync.wait_ge(s_out, 16)

        @block.scalar
        def _(scalar):
            # slabs 0-2 are small and needed first: keep them unthrottled
            # (only x block 0 competes); gate the big tail slabs behind them
            for s in range(min(3, N_SLABS)):
                slab_dma(scalar, s)
            if N_SLABS > 3:
                scalar.wait_ge(s_w[2], 16)
                for s in range(3, N_SLABS):
                    slab_dma(scalar, s)
            # preload ACT tables (Ln + Square + Identity) while DMAs fly
            scalar.activation(lnwarm[:, 0:1], nc.const_aps.tensor(1.0, (1, 1)), Ln)
            scalar.activation(lnwarm[:, 1:2], nc.const_aps.tensor(1.0, (1, 1)),
                              Square)
            scalar.activation(lnwarm[:, 2:3], nc.const_aps.tensor(1.0, (1, 1)),
                              Ident)
            scalar.wait_ge(s_pe, 1)
            scalar.activation(bufB[:], psB[:], Square,
                              scale=sactB[:2 * NBB]).then_inc(s_a)     # 1
            scalar.wait_ge(s_pe, 2)
            scalar.activation(bufA[:], psA[:], Square,
                              scale=sactA[:]).then_inc(s_a)            # 2
            scalar.wait_ge(s_v, 1)
            scalar.activation(lnm[:], m2c[:], Ln).then_inc(s_a)        # 3
            scalar.wait_ge(s_g, 1)
            scalar.activation(lnr[:], rall[:], Ln).then_inc(s_a)       # 4
            scalar.wait_ge(s_v, 3)
            for i in range(2):   # db_i = Identity(DB_SCALE*lnm + (-DB_SCALE*lnr_i))
                scalar.activation(db[:, i * T:(i + 1) * T],
                                  lnm[:, i * T:(i + 1) * T], Ident,
                                  bias=nlnr[:, i:i + 1],
                                  scale=float(DB_SCALE)).then_inc(s_a)  # 5, 6

        @block.vector
        def _(vector):
            vector.wait_ge(s_pe, 3)
            vector.tensor_scalar_max(m2c[:], psM[:],
                                     float(AMIN) ** 2).then_inc(s_v)   # 1
            vector.tensor_reduce(r1u[:], psM.rearrange("p (i f) -> p i f", i=NI),
                                 axis=mybir.AxisListType.X,
                                 op=mybir.AluOpType.max).then_inc(s_v)  # 2
            vector.wait_ge(s_a, 4)
            vector.tensor_scalar_mul(nlnr[:], lnr[:],
                                     -float(DB_SCALE)).then_inc(s_v)   # 3
            for i in range(2, 4):
                vector.tensor_scalar(db[:, i * T:(i + 1) * T],
                                     lnm[:, i * T:(i + 1) * T],
                                     lnr[:, i:i + 1], float(DB_SCALE),
                                     mybir.AluOpType.subtract,
                                     mybir.AluOpType.mult)
            vector.drain().then_inc(s_v)                                # 4

        @block.gpsimd
        def _(gpsimd):
            x_dma(gpsimd, 1)    # (j0, r0) items 2-3
            gpsimd.wait_ge(s_v, 2)
            gpsimd.partition_all_reduce(rall[:], r1u[:], channels=N_BINS,
                                        reduce_op=bass_isa.ReduceOp.max
                                        ).then_inc(s_g)                # 1
            gpsimd.wait_ge(s_v, 4)
            gpsimd.dma_start(outf[:, 2 * T:], db[:, 2 * T:]).then_inc(s_out2, 16)
            if not NOWAIT:
                gpsimd.wait_ge(s_out2, 16)

        @block.tensor
        def _(tensor):
            # HAM warmup on whatever garbage sits in junk's SBUF region:
            # psW is never read and every real accumulation opens with
            # start=True, so the values are irrelevant - starting the PE
            # immediately buys clock-ramp time
            for n in (NT, NT, NT, NT, NT, NT // 2):
                tensor.matmul(psW[:, :n], lhsT=junk[:, :P], rhs=junk[:, :n],
                              start=True, stop=True)
            waited = set()

            def need(sem):
                if id(sem) not in waited:
                    tensor.wait_ge(sem, 16)
                    waited.add(id(sem))

            na = sum(1 for e in SCHEDULE if e["kind"] == "A")
            nb = NE - na
            na_seen = nb_seen = 0
            for e, ent in enumerate(SCHEDULE):
                for sl in (ent["s0"], ent["s1"]):
                    for k in _xpieces(*sl):
                        need(s_x[k])
                need(s_w[_slab_of(e)])
                m = ent["m"]
                ps = psA if ent["kind"] == "A" else psB
                first = (na_seen == 0) if ent["kind"] == "A" else (nb_seen == 0)
                last = (na_seen == na - 1) if ent["kind"] == "A" \
                    else (nb_seen == nb - 1)
                tensor.matmul(ps[:m], lhsT=lhs_for(e), rhs=rhs_for(ent),
                              start=first, stop=last, perf_mode=DR,
                              skip_group_check=True)
                if ent["kind"] == "A":
                    na_seen += 1
                    if na_seen == na:
                        tensor.drain().then_inc(s_pe, 1)       # 2 (B first)
                else:
                    nb_seen += 1
                    if nb_seen == nb:
                        tensor.drain().then_inc(s_pe, 1)       # 1
            # pair-sum matmuls: psM[k] = buf[2k] + buf[2k+1] = re^2 + im^2
            tensor.wait_ge(s_a, 1)
            tensor.matmul(psM[SPLIT_BIN:], lhsT=SBv[:2 * NBB], rhs=bufB[:],
                          start=True, stop=True, skip_group_check=True)
            tensor.wait_ge(s_a, 2)
            tensor.matmul(psM[:SPLIT_BIN], lhsT=SAv[:], rhs=bufA[:],
                          start=True, stop=True, skip_group_check=True)
            tensor.drain().then_inc(s_pe, 1)                   # 3

    nc.compile()
    return nc


_PROGRAM = None


def _get_program():
    global _PROGRAM
    if _PROGRAM is None:
        _PROGRAM = build_program()
    return _PROGRAM


def run(x, **spmd_kwargs):
    """Run on 8 NeuronCores; returns (output [32, 84, 126] f32, results)."""
    nc = _get_program()
    packs = pack_x(x)
    in_maps = [{"x_in": packs[i], "w_in": W_NP} for i in range(N_CORES)]
    res = run_bass_kernel_spmd(nc, in_maps, core_ids=list(range(N_CORES)),
                               **spmd_kwargs)
    out = np.concatenate([res.results[i]["out"].transpose(1, 0, 2)
                          for i in range(N_CORES)], axis=0)
    return np.ascontiguousarray(out.astype(np.float32)), res


def kernel(x):
    return run(x)[0]


# revision 59
# speedup vs baseline: 1.3111x; 1.0547x over previous
"""CQT (constant-Q transform) + amplitude_to_db kernel for Trainium2.

Full-input contract: kernel(x) takes x [32, 64000] f32 and returns
[32, 84, 126] f32, matching:

    frames = pad(x, n_fft//2)[:, t*HOP + n]          # [B, 126, 16384]
    cr/ci  = frames @ Kr.T / Ki.T                    # [B, 84, 126]
    mag    = sqrt(cr^2 + ci^2)
    out    = amplitude_to_db(mag, ref=max per item, amin=1e-5, top_db=80)

Sharding: pure data parallelism - 4 batch items per NeuronCore on 8 cores.

v5: fp8e4 (e4m3) DoubleRow matmuls; one instruction contracts TWO
128-row K-chunk slots (lhsT [128,2,M], rhs [128,2,N]) in the 504 cycles
a single fp16 matmul costs, halving PE time vs fp16.

Sparsity: CQT kernel support halves per octave and is centered, so for
each K-chunk only a PREFIX of bins (lowest ones) is nonzero. Bins are
packed (re,im)-interleaved along psum partitions so each DoubleRow slot
ships only its active prefix (variable stationary width M_e) - about
4x fewer weight bytes than dense 128-wide slots. m2 = re^2+im^2 is then
formed by a tiny pair-summing matmul (S[p,m]=1 iff p//2==m) on the PE.

Precision: single-rounded fp8 pairs give rel_l2 ~1.8e-2 (gate is 2e-2).
The 16 highest-energy (group, chunk) slots instead use error-feedback
"comp" entries: Q0=q(W/2), Q1=q(W-Q0), x0=q(16x), x1=q(32x-x0) and the
two j-slots compute Q0.T x0 + Q1.T x1, halving both quantization noises
where it matters: measured rel_l2 ~1.0e-2.

Per-bin power-of-2 weight scales keep e4m3 in its normal range; the
epilogue unscales for free via ACT per-partition scale operands. GpSimd
runs ONLY the partition all-reduce + one output DMA (element-wise ops
there force a ~7us mid-kernel GpSimd library swap). DMA issues are
gated in need-order: HBM bandwidth is round-robin across in-flight
queues, so ungated late transfers starve the early ones.
"""

import os
import numpy as np
import ml_dtypes

import concourse.bass as bass
import concourse.mybir as mybir
from concourse import bacc
from concourse import bass_isa
from concourse.bass_utils import run_bass_kernel_spmd

# ---- problem constants (hardcoded; must match the reference) ----
SR = 22050
HOP = 512
N_BINS = 84
BPO = 12
FMIN = 32.70319566257483
AMIN = 1e-5
TOP_DB = 80.0
B = 32
N_SAMP = 64000
N_CORES = 8
NI = B // N_CORES            # items per core = 4
T = 1 + N_SAMP // HOP        # 126 frames
NT = NI * T                  # 504
DB_SCALE = 10.0 / np.log(10.0)
P = 128
SPLIT_BIN = 64               # group A: bins [0,64), group B: bins [64,84)
NBB = N_BINS - SPLIT_BIN     # 20
X_SCALE = 16.0
F8 = ml_dtypes.float8_e4m3   # == mybir.dt.float8e4

SCHEME = os.environ.get("CQT_SCHEME", "hybrid")   # hybrid | comp | pairs
# Skip the final output-DMA semaphore waits (teardown then overlaps the
# output DMA). Measured neutral-to-slightly-worse, so off by default.
NOWAIT = os.environ.get("CQT_NOWAIT", "0") == "1"


def _build_cqt_kernels():
    """Same construction as the reference (nnAudio-style direct CQT bank)."""
    Q = 1.0 / (2.0 ** (1.0 / BPO) - 1.0)
    freqs = FMIN * 2.0 ** (np.arange(N_BINS) / BPO)
    lengths = np.ceil(Q * SR / freqs).astype(int)
    n_fft = int(2 ** np.ceil(np.log2(lengths.max())))
    K = np.zeros((N_BINS, n_fft), dtype=np.complex128)
    for k in range(N_BINS):
        L = int(lengths[k])
        t = np.arange(L) - (L - 1) / 2.0
        kern = np.hanning(L) * np.exp(2j * np.pi * freqs[k] * t / SR)
        kern /= np.abs(kern).sum()
        kern /= np.sqrt(L)
        s = (n_fft - L) // 2
        K[k, s:s + L] = kern
    return K.real.astype(np.float32), K.imag.astype(np.float32), n_fft


Kr, Ki, N_FFT = _build_cqt_kernels()
PAD = N_FFT // 2
FW = (N_SAMP + 2 * PAD) // P      # 628
QW = FW // 4                      # 157
XB = 2 * NI * QW                  # 1256, one x-DMA block (2 phases x NI x QW)
assert (N_SAMP + 2 * PAD) % P == 0 and HOP == 4 * P

_NZ = (np.abs(Kr) + np.abs(Ki)) > 0


def _chunk_range(bins):
    nz = _NZ[bins].any(axis=0)
    idx = np.nonzero(nz)[0]
    return int(idx[0]) // P, int(idx[-1]) // P + 1

_A0, _A1 = _chunk_range(range(0, SPLIT_BIN))
_B0, _B1 = _chunk_range(range(SPLIT_BIN, N_BINS))
# Truncate the outermost Hann-tail chunks of group A: the outer 12 chunks
# per side hold ~1e-3 of the low bins' kernel energy, below the fp8
# quantization noise floor (simulated rel_l2 1.062e-2 vs 1.015e-2).
TRUNC = 12
CHUNKS_A = list(range(_A0 + TRUNC, _A1 - TRUNC))   # 74 chunks
CHUNKS_B = list(range(_B0, _B1))                   # 4 chunks


def _nbins(kind, c):
    """Active-bin count for (group, chunk); active bins are a prefix of
    the group (lowest bins have the widest support)."""
    lo, hi = (0, SPLIT_BIN) if kind == "A" else (SPLIT_BIN, N_BINS)
    act = np.nonzero(_NZ[lo:hi, c * P:(c + 1) * P].any(axis=1))[0]
    assert len(act) > 0 and act[-1] == len(act) - 1, (kind, c, act)
    return int(len(act))

# per-bin power-of-2 scales: peak |w| lands in [80, 160) (e4m3 max = 240)
_wmax = np.maximum(np.abs(Kr).max(axis=1), np.abs(Ki).max(axis=1))
SBIN = 2.0 ** np.floor(np.log2(160.0 / _wmax))

# comp set: top-16 (group, chunk) by filterbank energy (sim rel_l2 1.02e-2)
if SCHEME == "comp":
    COMP = {("A", c) for c in CHUNKS_A} | {("B", c) for c in CHUNKS_B}
elif SCHEME == "pairs":
    COMP = set()
else:
    COMP = ({("B", 63), ("B", 64)} |
            {("A", c) for c in range(58, 70)})


def _xoff(c, j):
    """Column offset of (chunk, j-variant) in the xt free layout
    (j, r, i, q): off = j*2512 + r*628 + i*157 + q."""
    return j * (4 * NI * QW) + (c % 4) * (NI * QW) + (c // 4)


def _xpieces(c, j):
    """x DMA pieces gating slice (c, j): the j0 phase-0 block is split
    in two item-halves on separate queues so the first matmuls can start
    as soon as ~2x40KB has landed in parallel."""
    if j == 0:
        r = c % 4
        if r == 0:
            return (0, 1)
        return (2,) if r == 1 else (3,)
    return (4 + (c % 4) // 2,)


def _xblk(c, j):
    """Max gating piece index, used only for schedule ordering."""
    return max(_xpieces(c, j))


# x DMA pieces as (col_lo, col_hi) of the xt free layout
HQW = NI * QW // 2   # 314: two items' worth of one phase
XPIECE = [(0, HQW), (HQW, 2 * HQW), (NI * QW, 2 * NI * QW), (XB, 2 * XB),
          (2 * XB, 3 * XB), (3 * XB, 4 * XB)]
N_XP = len(XPIECE)


def _build_schedule():
    """Entries: dict(kind, comp, s0, s1, m). Ordering: j0-only pair
    entries by x-block, then comp entries (need j1 blocks); B entries
    lead each segment; a few j0 pairs are held back to the end so the
    last B entry retires well before the last A entry."""
    ents = []
    for kind, chunks in (("A", CHUNKS_A), ("B", CHUNKS_B)):
        kord = 0 if kind == "B" else 1
        comp = [c for c in chunks if (kind, c) in COMP]
        rest = [c for c in chunks if (kind, c) not in COMP]
        for c in comp:
            ents.append(dict(kind=kind, comp=True, s0=(c, 0), s1=(c, 1),
                             m=2 * _nbins(kind, c),
                             key=(_xblk(c, 1), kord, c % 4, c // 4)))
        byphase = {}
        for c in rest:
            byphase.setdefault(c % 4, []).append(c)
        leftovers = []
        for r in sorted(byphase):
            lst = sorted(byphase[r])
            while len(lst) >= 2:
                c1, c2 = lst.pop(0), lst.pop(0)
                ents.append(dict(kind=kind, comp=False, s0=(c1, 0), s1=(c2, 0),
                                 m=2 * max(_nbins(kind, c1), _nbins(kind, c2)),
                                 key=(_xblk(c1, 0), kord, c1 % 4, c1 // 4)))
            leftovers += lst
        leftovers.sort(key=lambda c: _xoff(c, 0))
        while len(leftovers) >= 2:
            c1, c2 = leftovers.pop(0), leftovers.pop(0)
            blk = max(_xblk(c1, 0), _xblk(c2, 0))
            ents.append(dict(kind=kind, comp=False, s0=(c1, 0), s1=(c2, 0),
                             m=2 * max(_nbins(kind, c1), _nbins(kind, c2)),
                             key=(blk, kord, 5, 999)))
        if leftovers:   # odd count: upgrade the last single to a comp entry
            c = leftovers[0]
            ents.append(dict(kind=kind, comp=True, s0=(c, 0), s1=(c, 1),
                             m=2 * _nbins(kind, c),
                             key=(_xblk(c, 1), kord, 5, 999)))
    ents.sort(key=lambda e: e["key"])
    # hold back up to 4 j0-only A pairs to the very end (B-drain slack)
    tail = [e for e in ents if e["kind"] == "A" and not e["comp"]
            and e["key"][0] == 1][-4:]
    for e in tail:
        ents.remove(e)
    ents += tail
    assert ents[-1]["kind"] == "A"
    return ents


SCHEDULE = _build_schedule()
NE = len(SCHEDULE)

# header: sactA | sactB scale vectors (fp32), S_A and S_B pair-sum
# matrices (bf16), all bitcast into fp8 columns
HDR_SACTA = 0            # 4 cols  (fp32 [128,1])
HDR_SACTB = 4            # 4 cols
HDR_SA = 8               # 128 cols (bf16 [128, 64])
HDR_SB = 136             # 40 cols  (bf16 [.., 20])
HDR = 176

def _mpad(m):
    """DoubleRow fp8 LDWEIGHTS requires the j-slot stride to be even and
    16B-aligned (s3_lw_dual_fp8_restrictions)."""
    return (m + 15) // 16 * 16


WOFF = []
_off = HDR
for _e in SCHEDULE:
    WOFF.append(_off)
    _off += 2 * _mpad(_e["m"])
W_COLS = _off

# weight slab boundaries: by cumulative bytes (finer early)
_bfr = [0.03, 0.08, 0.18, 0.35, 0.6, 1.0]
SLAB_ENDS = []
prev = 0
for f in _bfr:
    target = HDR + f * (W_COLS - HDR)
    e = prev + 1
    while e < NE and WOFF[e] < target:
        e += 1
    e = min(e, NE)
    if e > prev:
        SLAB_ENDS.append(e)
        prev = e
SLAB_ENDS[-1] = NE
N_SLABS = len(SLAB_ENDS)


def _slab_of(e):
    for s, end in enumerate(SLAB_ENDS):
        if e < end:
            return s
    raise IndexError(e)


def _slab_cols(s):
    lo = 0 if s == 0 else WOFF[SLAB_ENDS[s - 1]] if SLAB_ENDS[s - 1] < NE else W_COLS
    hi = WOFF[SLAB_ENDS[s]] if SLAB_ENDS[s] < NE else W_COLS
    return lo, hi


def _pack_weights():
    q = lambda a: a.astype(F8)
    KrT = (Kr * SBIN[:, None]).T.astype(np.float32)   # [N_FFT, 84]
    KiT = (Ki * SBIN[:, None]).T.astype(np.float32)

    def chunk_w(kind, c, m):
        """Interleaved [128, m] block: col 2k = re(bin k), 2k+1 = im."""
        base = 0 if kind == "A" else SPLIT_BIN
        Wc = np.zeros((P, m), np.float32)
        nb = _nbins(kind, c)
        rows = slice(c * P, (c + 1) * P)
        Wc[:, 0:2 * nb:2] = KrT[rows, base:base + nb]
        Wc[:, 1:2 * nb:2] = KiT[rows, base:base + nb]
        return Wc

    w = np.zeros((P, W_COLS), F8)
    for e, ent in enumerate(SCHEDULE):
        base = WOFF[e]
        m = ent["m"]
        mp = _mpad(m)
        if ent["comp"]:
            Wc = chunk_w(ent["kind"], ent["s0"][0], m)
            Q0 = q(Wc * 0.5)
            Q1 = q(Wc - Q0.astype(np.float32))
            w[:, base:base + m] = Q0
            w[:, base + mp:base + mp + m] = Q1
        else:
            w[:, base:base + m] = q(chunk_w(ent["kind"], ent["s0"][0], m))
            w[:, base + mp:base + mp + m] = q(chunk_w(ent["kind"], ent["s1"][0], m))

    sinv = 1.0 / (SBIN * X_SCALE)
    sactA = np.ones(P, np.float32)
    sactA[:] = sinv[np.arange(P) // 2]                  # psA partition 2k/2k+1
    sactB = np.ones(P, np.float32)
    sactB[:2 * NBB] = sinv[SPLIT_BIN + np.arange(2 * NBB) // 2]
    SA = np.zeros((P, SPLIT_BIN), ml_dtypes.bfloat16)
    SA[np.arange(P), np.arange(P) // 2] = 1.0
    SB = np.zeros((P, NBB), ml_dtypes.bfloat16)
    SB[np.arange(2 * NBB), np.arange(2 * NBB) // 2] = 1.0
    wu8 = w.view(np.uint8)
    wu8[:, HDR_SACTA:HDR_SACTA + 4] = sactA.astype("<f4").view(np.uint8).reshape(P, 4)
    wu8[:, HDR_SACTB:HDR_SACTB + 4] = sactB.astype("<f4").view(np.uint8).reshape(P, 4)
    wu8[:, HDR_SA:HDR_SA + 2 * SPLIT_BIN] = SA.view(np.uint8).reshape(P, -1)
    wu8[:, HDR_SB:HDR_SB + 2 * NBB] = SB.view(np.uint8).reshape(P, -1)
    return w


W_NP = _pack_weights()


def pack_x(x):
    """x [32, 64000] f32 -> per-core fp8 packs [4, 128, 1256].

    SBUF xt free layout (j, r, i, q); DMA block k = 2j + r//2.
    x0 = q(16 x); x1 = q(32 x - x0)."""
    xs = np.asarray(x, dtype=np.float32)
    xp = np.pad(xs, ((0, 0), (PAD, PAD)))                 # [32, 80384]
    x0 = (xp * X_SCALE).astype(F8)
    x1 = (2.0 * X_SCALE * xp - x0.astype(np.float32)).astype(F8)
    # sample n = 512 q + 128 r + p  ->  [j, bi, q, r, p]
    X = np.stack([x0, x1]).reshape(2, B, QW, 4, P)
    X = X.reshape(2, B, QW, 2, 2, P)                      # [j, bi, q, rblk, rib, p]
    packs = []
    for core in range(N_CORES):
        blk = X[:, core * NI:(core + 1) * NI]             # [j, i, q, rblk, rib, p]
        arr = blk.transpose(0, 3, 5, 4, 1, 2)             # [j, rblk, p, rib, i, q]
        packs.append(np.ascontiguousarray(arr.reshape(4, P, XB)))
    return packs


def build_program():
    nc = bacc.Bacc("TRN2", target_bir_lowering=False, debug=False,
                   enable_asserts=True)
    f8 = mybir.dt.float8e4
    bf16 = mybir.dt.bfloat16
    f32 = mybir.dt.float32
    DR = mybir.MatmulPerfMode.DoubleRow
    Ln = mybir.ActivationFunctionType.Ln
    Square = mybir.ActivationFunctionType.Square
    Ident = mybir.ActivationFunctionType.Identity

    x_in = nc.dram_tensor("x_in", [4, P, XB], f8, kind="ExternalInput").ap()
    w_in = nc.dram_tensor("w_in", [P, W_COLS], f8, kind="ExternalInput").ap()
    out = nc.dram_tensor("out", [N_BINS, NI, T], f32, kind="ExternalOutput").ap()

    xt = nc.alloc_sbuf_tensor("xt", [P, 4 * XB], f8).ap()
    wt = nc.alloc_sbuf_tensor("wt", [P, W_COLS], f8).ap()
    junk = nc.alloc_sbuf_tensor("junk", [P, 512], f8).ap()
    bufA = nc.alloc_sbuf_tensor("bufA", [P, NT], bf16).ap()
    bufB = nc.alloc_sbuf_tensor("bufB", [2 * NBB, NT], bf16).ap()
    m2c = nc.alloc_sbuf_tensor("m2c", [N_BINS, NT], f32).ap()
    lnm = nc.alloc_sbuf_tensor("lnm", [N_BINS, NT], f32).ap()
    db = nc.alloc_sbuf_tensor("db", [N_BINS, NT], f32).ap()
    r1u = nc.alloc_sbuf_tensor("r1u", [N_BINS, NI], f32).ap()
    rall = nc.alloc_sbuf_tensor("rall", [N_BINS, NI], f32).ap()
    lnr = nc.alloc_sbuf_tensor("lnr", [N_BINS, NI], f32).ap()
    nlnr = nc.alloc_sbuf_tensor("nlnr", [N_BINS, NI], f32).ap()
    lnwarm = nc.alloc_sbuf_tensor("lnwarm", [1, 3], f32).ap()

    psW = nc.alloc_psum_tensor("psW", [P, NT], f32).ap()
    psA = nc.alloc_psum_tensor("psA", [P, NT], f32).ap()
    psB = nc.alloc_psum_tensor("psB", [2 * NBB, NT], f32).ap()
    psM = nc.alloc_psum_tensor("psM", [N_BINS, NT], f32).ap()

    sactA = wt[:, HDR_SACTA:HDR_SACTA + 4].bitcast(f32)    # [128, 1]
    sactB = wt[:, HDR_SACTB:HDR_SACTB + 4].bitcast(f32)
    SAv = wt[:, HDR_SA:HDR_SA + 2 * SPLIT_BIN].bitcast(bf16)   # [128, 64]
    SBv = wt[:, HDR_SB:HDR_SB + 2 * NBB].bitcast(bf16)         # [128, 20]

    s_x = [nc.alloc_semaphore(f"s_x{k}") for k in range(N_XP)]
    s_w = [nc.alloc_semaphore(f"s_w{s}") for s in range(N_SLABS)]
    s_mi = nc.alloc_semaphore("s_mi")
    s_pe = nc.alloc_semaphore("s_pe")   # 1 psB final, 2 psA final, 3 psM final
    s_a = nc.alloc_semaphore("s_a")     # scalar ACT steps
    s_v = nc.alloc_semaphore("s_v")     # vector steps
    s_g = nc.alloc_semaphore("s_g")     # gpsimd steps
    s_out = nc.alloc_semaphore("s_out")
    s_out2 = nc.alloc_semaphore("s_out2")

    xv = xt.rearrange("p (j r i q) -> p j r i q", j=2, r=4, i=NI)

    def rhs_for(ent):
        (c1, j1), (c2, j2) = ent["s0"], ent["s1"]
        o1, o2 = _xoff(c1, j1), _xoff(c2, j2)
        assert o2 > o1, (c1, j1, c2, j2)
        base = xv[:, j1, c1 % 4, :, (c1 // 4):(c1 // 4) + T]   # [128, NI, T]
        u = base.unsqueeze(1)
        u.ap[1] = [o2 - o1, 2]                                  # [128, 2, NI, T]
        return u

    def lhs_for(e):
        m = SCHEDULE[e]["m"]
        lo = WOFF[e]
        u = wt[:, lo:lo + m].unsqueeze(1)
        u.ap[1] = [_mpad(m), 2]       # [128, 2, m] with 16B-aligned j stride
        return u

    def slab_dma(eng, s):
        lo, hi = _slab_cols(s)
        eng.dma_start(wt[:, lo:hi], w_in[:, lo:hi]).then_inc(s_w[s], 16)

    def x_dma(eng, k):
        lo, hi = XPIECE[k]
        if k < 3:
            src = x_in[0][:, lo:hi]
        else:
            src = x_in[k - 2]
        eng.dma_start(xt[:, lo:hi], src).then_inc(s_x[k], 16)

    outf = out.rearrange("k i t -> k (i t)")

    with nc.Block() as block:

        # DMA choreography: HBM bandwidth is shared round-robin across all
        # in-flight hardware queues, so late big transfers are gated behind
        # the early small ones (ungated they starve the early slabs, which
        # stalls the PE and resets the HAM clock ramp).
        @block.sync
        def _(sync):
            x_dma(sync, 0)      # (j0, r0) items 0-1; items 2-3 on gpsimd
            x_dma(sync, 2)      # (j0, r1)
            sync.wait_ge(s_x[2], 16)
            for k in range(3, N_XP):
                x_dma(sync, k)
            sync.wait_ge(s_a, 6)
            sync.dma_start(outf[:, :2 * T], db[:, :2 * T]).then_inc(s_out, 16)
            if not NOWAIT:
                sync.wait_ge(s_out, 16)

        @block.scalar
        def _(scalar):
            # slabs 0-2 are small and needed first: keep them unthrottled
            # (only x block 0 competes); gate the big tail slabs behind them
            for s in range(min(3, N_SLABS)):
                slab_dma(scalar, s)
            if N_SLABS > 3:
                scalar.wait_ge(s_w[2], 16)
                for s in range(3, N_SLABS):
                    slab_dma(scalar, s)
            # preload ACT tables (Ln + Square + Identity) while DMAs fly
            scalar.activation(lnwarm[:, 0:1], nc.const_aps.tensor(1.0, (1, 1)), Ln)
            scalar.activation(lnwarm[:, 1:2], nc.const_aps.tensor(1.0, (1, 1)),
                              Square)
            scalar.activation(lnwarm[:, 2:3], nc.const_aps.tensor(1.0, (1, 1)),
                              Ident)
            scalar.wait_ge(s_pe, 1)
            scalar.activation(bufB[:], psB[:], Square,
                              scale=sactB[:2 * NBB]).then_inc(s_a)     # 1
            scalar.wait_ge(s_pe, 2)
            scalar.activation(bufA[:], psA[:], Square,
                              scale=sactA[:]).then_inc(s_a)            # 2
            scalar.wait_ge(s_v, 1)
            scalar.activation(lnm[:], m2c[:], Ln).then_inc(s_a)        # 3
            scalar.wait_ge(s_g, 1)
            scalar.activation(lnr[:], rall[:], Ln).then_inc(s_a)       # 4
            scalar.wait_ge(s_v, 3)
            for i in range(2):   # db_i = Identity(DB_SCALE*lnm + (-DB_SCALE*lnr_i))
                scalar.activation(db[:, i * T:(i + 1) * T],
                                  lnm[:, i * T:(i + 1) * T], Ident,
                                  bias=nlnr[:, i:i + 1],
                                  scale=float(DB_SCALE)).then_inc(s_a)  # 5, 6

        @block.vector
        def _(vector):
            vector.memset(junk[:], 0.0).then_inc(s_mi, 1)
            vector.wait_ge(s_pe, 3)
            vector.tensor_scalar_max(m2c[:], psM[:],
                                     float(AMIN) ** 2).then_inc(s_v)   # 1
            vector.tensor_reduce(r1u[:], psM.rearrange("p (i f) -> p i f", i=NI),
                                 axis=mybir.AxisListType.X,
                                 op=mybir.AluOpType.max).then_inc(s_v)  # 2
            vector.wait_ge(s_a, 4)
            vector.tensor_scalar_mul(nlnr[:], lnr[:],
                                     -float(DB_SCALE)).then_inc(s_v)   # 3
            for i in range(2, 4):
                vector.tensor_scalar(db[:, i * T:(i + 1) * T],
                                     lnm[:, i * T:(i + 1) * T],
                                     lnr[:, i:i + 1], float(DB_SCALE),
                                     mybir.AluOpType.subtract,
                                     mybir.AluOpType.mult)
            vector.drain().then_inc(s_v)                                # 4

        @block.gpsimd
        def _(gpsimd):
            x_dma(gpsimd, 1)    # (j0, r0) items 2-3
            gpsimd.wait_ge(s_v, 2)
            gpsimd.partition_all_reduce(rall[:], r1u[:], channels=N_BINS,
                                        reduce_op=bass_isa.ReduceOp.max
                                        ).then_inc(s_g)                # 1
            gpsimd.wait_ge(s_v, 4)
            gpsimd.dma_start(outf[:, 2 * T:], db[:, 2 * T:]).then_inc(s_out2, 16)
            if not NOWAIT:
                gpsimd.wait_ge(s_out2, 16)

        @block.tensor
        def _(tensor):
            # HAM warmup on whatever garbage sits in junk's SBUF region
            # (psW is never read) - starting the PE immediately buys
            # clock-ramp time. The psA/psB matmuls also multiply garbage
            # but carry start=True, zero-initializing every partition so
            # the variable-width entries (which only touch their prefix
            # partitions) can all use start=False... they must multiply
            # ZEROS, so junk is first cleared by the one matmul-sized
            # memset-free trick: lhsT reads the zeroed const region.
            for n in (NT, NT, NT):
                tensor.matmul(psW[:, :n], lhsT=junk[:, :P], rhs=junk[:, :n],
                              start=True, stop=True)
            tensor.wait_ge(s_mi, 1)   # junk zeroed during the warmups above
            tensor.matmul(psA[:], lhsT=junk[:, :P], rhs=junk[:, :NT],
                          start=True, stop=False, skip_group_check=True)
            tensor.matmul(psB[:], lhsT=junk[:, :2 * NBB], rhs=junk[:, :NT],
                          start=True, stop=False, skip_group_check=True)
            tensor.matmul(psW[:, :NT // 2], lhsT=junk[:, :P],
                          rhs=junk[:, :NT // 2], start=True, stop=True)
            waited = set()

            def need(sem):
                if id(sem) not in waited:
                    tensor.wait_ge(sem, 16)
                    waited.add(id(sem))

            na = sum(1 for e in SCHEDULE if e["kind"] == "A")
            nb = NE - na
            na_seen = nb_seen = 0
            for e, ent in enumerate(SCHEDULE):
                for sl in (ent["s0"], ent["s1"]):
                    for k in _xpieces(*sl):
                        need(s_x[k])
                need(s_w[_slab_of(e)])
                m = ent["m"]
                ps = psA if ent["kind"] == "A" else psB
                last = (na_seen == na - 1) if ent["kind"] == "A" \
                    else (nb_seen == nb - 1)
                tensor.matmul(ps[:m], lhsT=lhs_for(e), rhs=rhs_for(ent),
                              start=False, stop=last, perf_mode=DR,
                              skip_group_check=True)
                if ent["kind"] == "A":
                    na_seen += 1
                    if na_seen == na:
                        tensor.drain().then_inc(s_pe, 1)       # 2 (B first)
                else:
                    nb_seen += 1
                    if nb_seen == nb:
                        tensor.drain().then_inc(s_pe, 1)       # 1
            # pair-sum matmuls: psM[k] = buf[2k] + buf[2k+1] = re^2 + im^2
            tensor.wait_ge(s_a, 1)
            tensor.matmul(psM[SPLIT_BIN:], lhsT=SBv[:2 * NBB], rhs=bufB[:],
                          start=True, stop=True, skip_group_check=True)
            tensor.wait_ge(s_a, 2)
            tensor.matmul(psM[:SPLIT_BIN], lhsT=SAv[:], rhs=bufA[:],
                          start=True, stop=True, skip_group_check=True)
            tensor.drain().then_inc(s_pe, 1)                   # 3

    nc.compile()
    return nc


_PROGRAM = None


def _get_program():
    global _PROGRAM
    if _PROGRAM is None:
        _PROGRAM = build_program()
    return _PROGRAM


def run(x, **spmd_kwargs):
    """Run on 8 NeuronCores; returns (output [32, 84, 126] f32, results)."""
    nc = _get_program()
    packs = pack_x(x)
    in_maps = [{"x_in": packs[i], "w_in": W_NP} for i in range(N_CORES)]
    res = run_bass_kernel_spmd(nc, in_maps, core_ids=list(range(N_CORES)),
                               **spmd_kwargs)
    out = np.concatenate([res.results[i]["out"].transpose(1, 0, 2)
                          for i in range(N_CORES)], axis=0)
    return np.ascontiguousarray(out.astype(np.float32)), res


def kernel(x):
    return run(x)[0]
